# revision 37
# baseline (speedup 1.0000x reference)
"""CRvNN forward kernel for 8x Trainium2 NeuronCores (Bass/Tile).

Strategy
--------
Pure data parallelism: batch 32 -> 4 per core; params replicated; no
collectives.  On-device state lives in TRANSPOSED layout (D=256 on partitions
as 2x128 chunks, sequence position i on the free axis, padded 514 -> 516).

The reference's (S2 x S2) neighbor-probability matrices are first-order
linear recurrences; each (lnp @ x) / (rnp @ x) is ONE DVE tensor_tensor_scan
per 128-partition chunk.  No S^2 matrices, no PE transposes.

v2: fp16 datapath (TensorTensor 2x / tensor_scalar 4x DVE modes, halved DMA,
1 cyc/row matmuls).  The scan coefficient na=1-a stays fp32 (fp16 rounding
would compound over up-to-514 factors); row math stays fp32.

v3: batched row pipeline.  A (1,516) DVE op costs the same as (128,516), so
all per-batch row math runs once per step on (4,SP)/(12,SP) tiles.  Per-batch
LN/score sums land in a shared (12,SP) PSUM via one-hot lhsT columns
(m rows 0-3, v rows 4-7, tsc rows 8-11), copied out by a single ACT op with a
per-partition scale column.  Bounce-buffer DMA writes drop to 4 per step.

This walrus build supports only ONE sync wait per instruction; a
post-scheduling pass splits multi-wait instructions into single-wait NOP
chains.
"""
import os
import sys
from contextlib import ExitStack

import numpy as np

sys.path.insert(0, "/opt/trn_rl_repo")

import bass_rust
import concourse.bass as bass
import concourse.mybir as mybir
from concourse.tile import TileContext

F32 = mybir.dt.float32
F16 = mybir.dt.float16
U8 = mybir.dt.uint8
FP8 = mybir.dt.float8e4
AL = mybir.AluOpType
AF = mybir.ActivationFunctionType
PM = mybir.MatmulPerfMode

NCORES = 8
NB = 4            # batch per core
D = 256
DC = 2            # D chunks of 128
S2 = 514
SP = 516          # padded sequence length
SPP = SP + 2      # scan-input tiles have leading+trailing zero pad columns
H = 1024
WIN = 5
EPS = 1e-9

SIM = os.environ.get("CRVNN_SIM", "0") == "1"
TRACE = os.environ.get("CRVNN_TRACE", "0") == "1"
GP_LVL = int(os.environ.get("CRVNN_GP", "0"))
W1_FP8 = os.environ.get("CRVNN_W1", "fp8") == "fp8"
W2_FP8 = os.environ.get("CRVNN_W2", "fp8") == "fp8"
WSCALE = 64.0

NSPLITS = [(0, 512), (512, SP - 512)]

LAST_EXEC_NS = None
LAST_RES = None
LAST_NC = None


# --------------------------------------------------------------------------
# post-scheduling fixup: split multi-wait instructions into 1-wait NOP chains
# --------------------------------------------------------------------------
def _split_multiwaits(nc):
    counter = [0]

    def mk_nop(engine, wait):
        counter[0] += 1
        n = bass_rust.InstNoOp(name=f"WFIX-{counter[0]}", ins=[], outs=[])
        n.engine = engine
        n.sync_info = bass_rust.SyncInfo(on_wait=[wait], on_update=[])
        return n

    total = 0
    for f in nc.m.functions:
        for bb in f.blocks:
            out = []
            changed = False
            for inst in list(bb.instructions):
                si = inst.sync_info
                waits = list(si.on_wait) if (si is not None and si.on_wait) else []
                if len(waits) > 1:
                    for w in waits[:-1]:
                        out.append(mk_nop(inst.engine, w))
                    inst.sync_info = bass_rust.SyncInfo(
                        on_wait=[waits[-1]],
                        on_update=list(si.on_update) if si.on_update else [])
                    changed = True
                    total += 1
                out.append(inst)
            if changed:
                bb.instructions = out
    return total


def _bcast_ap(drow):
    """DRAM row AP (1, n) -> partition-broadcast AP (128, n)."""
    return bass.AP(tensor=drow.tensor, offset=drow.offset,
                   ap=[[0, 128]] + drow.ap[1:])


def _build_program(n_steps):
    nc = bass.Bass()

    seqT_in = nc.declare_dram_parameter("seqT", [NB, DC, 128, SP], F16, isOutput=False)
    mask_in = nc.declare_dram_parameter("mask", [NB, SP], F32, isOutput=False)
    selp_in = nc.declare_dram_parameter("selp", [NB, SP], F32, isOutput=False)
    a16_in = nc.declare_dram_parameter("a16", [NB, SP], F16, isOutput=False)
    na32_in = nc.declare_dram_parameter("na32", [NB, SP], F32, isOutput=False)
    itW_in = nc.declare_dram_parameter("itW", [D, D], F16, isOutput=False)
    convW_in = nc.declare_dram_parameter("convW", [WIN * D, D], F16, isOutput=False)
    scw6_in = nc.declare_dram_parameter("scw6", [NB, 128, DC, 66], F16,
                                        isOutput=False)
    if W1_FP8:
        w1W_in = nc.declare_dram_parameter("w1p", [2, 128, 2, H], U8,
                                           isOutput=False)
    else:
        w1W_in = nc.declare_dram_parameter("w1W", [2 * D, H], F16, isOutput=False)
    if W2_FP8:
        w2W_in = nc.declare_dram_parameter("w2p", [4, 128, 2, 4 * D], U8,
                                           isOutput=False)
    else:
        w2W_in = nc.declare_dram_parameter("w2W", [H, 4 * D], F16, isOutput=False)
    noc_in = nc.declare_dram_parameter("noc", [128, DC], F32, isOutput=False)
    ymn_in = nc.declare_dram_parameter("ymnc", [128, DC], F32, isOutput=False)
    out_dram = nc.declare_dram_parameter("out", [NB, DC, 128, S2], F16, isOutput=True)

    with TileContext(nc) as tc, ExitStack() as ctx:
        wpool = ctx.enter_context(tc.tile_pool(name="wpool", bufs=1))
        state = ctx.enter_context(tc.tile_pool(name="state", bufs=1))
        work = ctx.enter_context(tc.tile_pool(name="work", bufs=1))
        psum = ctx.enter_context(tc.tile_pool(name="psum", bufs=1, space="PSUM"))
        dram = ctx.enter_context(tc.tile_pool(name="dramp", bufs=1, space="DRAM"))

        # ---------------- weights -> SBUF (fp16, direct DMA) ----------------
        def load_w(name, dram_ap, shape, q=None):
            t = wpool.tile(shape, F16, name=name)
            (q or nc.scalar).dma_start(out=t, in_=dram_ap)
            return t

        convW_t = [load_w(f"convW{k}", convW_in.ap()[k * 128:(k + 1) * 128, :],
                          [128, D]) for k in range(10)]
        def load_u8(name, dram_ap, shape):
            t = wpool.tile(shape, U8, name=name)
            nc.scalar.dma_start(out=t, in_=dram_ap)
            return t

        if W1_FP8:
            w1W_t = [load_u8(f"w1p{p}", w1W_in.ap()[p], [128, 2, H])
                     for p in range(2)]
        else:
            w1W_t = [load_w(f"w1W{k}", w1W_in.ap()[k * 128:(k + 1) * 128, :],
                            [128, H]) for k in range(4)]
        if W2_FP8:
            w2W_t = [load_u8(f"w2p{p}", w2W_in.ap()[p], [128, 2, H])
                     for p in range(4)]
        else:
            w2W_t = [load_w(f"w2W{k}", w2W_in.ap()[k * 128:(k + 1) * 128, :],
                            [128, H]) for k in range(8)]
        scw6_t = [load_w(f"scw6_{b}", scw6_in.ap()[b], [128, DC, 66])
                  for b in range(NB)]
        itW_t = [load_w(f"itW{k}", itW_in.ap()[k * 128:(k + 1) * 128, :],
                        [128, D], q=nc.sync) for k in range(2)]

        noc = wpool.tile([128, DC], F32)
        nc.sync.dma_start(out=noc, in_=noc_in.ap())
        ymnc = wpool.tile([128, DC], F32)
        nc.sync.dma_start(out=ymnc, in_=ymn_in.ap())
        eps_t = wpool.tile([128, 1], F32)
        nc.vector.memset(eps_t, 1e-5)
        ones16 = wpool.tile([128, SP], F16)
        nc.vector.memset(ones16, 1.0)

        # one-hot stat lhsT columns (group-local, 32-aligned stat rows):
        # onesm[b] col b%2 = 1 (mean), onesv[b] col 32+b%2 = 1 (meansq);
        # score lhsT (scw66) has col 64+b%2 = scW.  Engine SBUF/PSUM accesses
        # must start at 32-aligned partitions, hence the spread layout.
        onesm, onesv = [], []
        for b in range(NB):
            gb = b % 2
            tm = wpool.tile([128, 66], F16, name=f"onesm{b}")
            nc.vector.memset(tm, 0.0)
            nc.vector.memset(tm[:, gb:gb + 1], 1.0)
            onesm.append(tm)
            tv = wpool.tile([128, 66], F16, name=f"onesv{b}")
            nc.vector.memset(tv, 0.0)
            nc.vector.memset(tv[:, 32 + gb:33 + gb], 1.0)
            onesv.append(tv)


        # ---------------- persistent state ----------------------------------
        seqT = [state.tile([128, DC, SP], F16, name=f"seqT{b}") for b in range(NB)]
        compS = [state.tile([128, DC, SP], F16, name=f"compS{b}") for b in range(NB)]
        a_row2 = [state.tile([2, SP], F32, name=f"a_row2{g}") for g in range(2)]
        mask2 = [state.tile([2, SP], F32, name=f"mask2{g}") for g in range(2)]
        selp2 = [state.tile([2, SP], F32, name=f"selp2{g}") for g in range(2)]
        for g in range(2):
            nc.sync.dma_start(out=a_row2[g], in_=mask_in.ap()[2 * g:2 * g + 2])
            nc.sync.dma_start(out=mask2[g], in_=mask_in.ap()[2 * g:2 * g + 2])
            nc.sync.dma_start(out=selp2[g], in_=selp_in.ap()[2 * g:2 * g + 2])
        nap2 = [state.tile([2, SPP], F32, name=f"nap2{g}") for g in range(2)]
        tpp2 = [state.tile([2, SPP], F32, name=f"tpp2{g}") for g in range(2)]
        for g in range(2):
            nc.vector.memset(nap2[g][:, 0:SPP:SPP - 1], 0.0)
            nc.vector.memset(tpp2[g][:, 0:SPP:SPP - 1], 0.0)

        # DRAM bounce tiles for partition-broadcast
        a4_d = dram.tile([4, SP], F16, name="a4_d")
        na4_d = dram.tile([4, SP], F32, name="na4_d")
        ltp4_d = dram.tile([4, SP], F16, name="ltp4_d")
        r3_d = dram.tile([12, SP], F16, name="r3_d")
        r2i_d = dram.tile([8, SP], F16, name="r2i_d")

        def work_big(name, tag, dtype=F16, bufs=None):
            return work.tile([128, DC, SP], dtype, name=name, tag=tag, bufs=bufs)

        def row4(name, dtype=F32):
            return work.tile([4, SP], dtype, name=name, tag="rowW", bufs=10)

        def tiny4(name):
            return work.tile([4, 1], F32, name=name, tag="tinyW", bufs=6)

        def bc_tile(name, dtype=F16):
            return work.tile([128, SP], dtype, name=name, tag="bcast", bufs=12)

        def bcast_read(drow_ap, name):
            t = bc_tile(name)
            nc.sync.dma_start(out=t, in_=_bcast_ap(drow_ap))
            return t

        def recip(out_r, in_r):
            nc.vector.reciprocal(out=out_r, in_=in_r)

        def tt(out, in0, in1, op, gp=False):
            eng = nc.gpsimd if (gp and GP_LVL > 0) else nc.vector
            eng.tensor_tensor(out=out, in0=in0, in1=in1, op=op)

        def mm_dr(psum_ap, wpairs, rhs_pairs, col0, ncols):
            """fp8 DoubleRow: contract pairs of 128-k-chunks per instruction.
            wpairs: uint8 [128,2,M] tiles; rhs_pairs: fp8 [128,2,SP] APs."""
            P = len(wpairs)
            for (o, s) in NSPLITS:
                for p in range(P):
                    nc.tensor.matmul(
                        psum_ap[:, o:o + s],
                        wpairs[p].bitcast(FP8)[:, :, col0:col0 + ncols],
                        rhs_pairs[p][:, :, o:o + s],
                        start=(p == 0), stop=(p == P - 1),
                        perf_mode=PM.DoubleRow)

        def mm(psum_ap, lhsT, rhs_chunks, nsl=NSPLITS):
            K = len(lhsT)
            for (o, s) in nsl:
                for k in range(K):
                    nc.tensor.matmul(psum_ap[:, o:o + s], lhsT[k],
                                     rhs_chunks[k][:, o:o + s],
                                     start=(k == 0), stop=(k == K - 1))

        def gelu_act(out, in_, scale=1.0):
            if SIM:
                x2 = work.tile([out.shape[0], out.shape[-1]], F32, name="gx2",
                               tag="gelu_tmp", bufs=2)
                nc.scalar.activation(out=x2, in_=in_, func=AF.Square, bias=0.0,
                                     scale=scale)
                nc.vector.tensor_scalar(out=x2, in0=x2, scalar1=0.044715,
                                        scalar2=1.0, op0=AL.mult, op1=AL.add)
                u = work.tile([out.shape[0], out.shape[-1]], F32, name="gu",
                              tag="gelu_tmp2")
                nc.scalar.activation(out=u, in_=in_, func=AF.Copy, scale=scale)
                nc.vector.tensor_tensor(out=x2, in0=x2, in1=u, op=AL.mult)
                nc.scalar.activation(out=x2, in_=x2, func=AF.Tanh,
                                     scale=0.7978845608028654)
                nc.vector.tensor_scalar(out=x2, in0=x2, scalar1=1.0,
                                        scalar2=0.5, op0=AL.add, op1=AL.mult)
                nc.vector.tensor_tensor(out=out, in0=x2, in1=u, op=AL.mult)
            else:
                nc.scalar.activation(out=out, in_=in_, func=AF.Gelu_apprx_tanh,
                                     bias=0.0, scale=scale)

        def scan_fwd(out_c, nap, datap):
            """out[i] = data[i-1] + na[i-1]*out[i-1]; data pad supplies z0=0."""
            nc.vector.tensor_tensor_scan(
                out=out_c, data0=nap[:, 0:SP], data1=datap[:, 0:SP],
                initial=0.0, op0=AL.mult, op1=AL.add)

        def scan_bwd(out_c, nap, datap):
            nc.vector.tensor_tensor_scan(
                out=out_c[:, ::-1], data0=nap[:, SPP - 1:1:-1],
                data1=datap[:, SPP - 1:1:-1], initial=0.0,
                op0=AL.mult, op1=AL.add)

        def mm_acc(psum_ap, lhsT, rhs_chunks, first=False, last=False):
            """Matmuls into a shared accumulation group: only the very first
            call (per split region) zeroes PSUM, only the last closes it."""
            K = len(lhsT)
            for (o, s) in NSPLITS:
                for k in range(K):
                    nc.tensor.matmul(psum_ap[:, o:o + s], lhsT[k],
                                     rhs_chunks[k][:, o:o + s],
                                     start=(first and k == 0),
                                     stop=(last and k == K - 1),
                                     skip_group_check=True)

        def stat_mms(ps6, b, src_big, last=False):
            """Accumulate batch b's LN sums into its group's stats psum."""
            mm_acc(ps6, [onesm[b], onesm[b]],
                   [src_big[:, 0, :], src_big[:, 1, :]])
            sq = [work.tile([128, SP], F16, name=f"sq{c}", tag="sq", bufs=2)
                  for c in range(DC)]
            for c in range(DC):
                nc.scalar.activation(out=sq[c], in_=src_big[:, c, :],
                                     func=AF.Square, bias=0.0)
            mm_acc(ps6, [onesv[b], onesv[b]], [sq[0], sq[1]], last=last)

        def ln_rows2(ps6, want_tsc=True):
            """(66,SP) psum (m 0-1, v 32-33, tsc 64-65) -> base-0 rows."""
            m2 = row4("m2")[0:2]
            v2 = row4("v2")[0:2]
            tsc2 = row4("tsc2")[0:2] if want_tsc else None
            for (o, s) in NSPLITS:
                nc.scalar.activation(out=m2[:, o:o + s], in_=ps6[0:2, o:o + s],
                                     func=AF.Copy, scale=1.0 / D)
                nc.scalar.activation(out=v2[:, o:o + s], in_=ps6[32:34, o:o + s],
                                     func=AF.Copy, scale=1.0 / D)
                if want_tsc:
                    nc.scalar.activation(out=tsc2[:, o:o + s],
                                         in_=ps6[64:66, o:o + s], func=AF.Copy)
            msq = row4("msq")[0:2]
            nc.vector.tensor_tensor(out=msq, in0=m2, in1=m2, op=AL.mult)
            var = row4("var")[0:2]
            nc.vector.tensor_tensor(out=var, in0=v2, in1=msq, op=AL.subtract)
            nc.scalar.activation(out=var, in_=var, func=AF.Sqrt,
                                 bias=eps_t[0:2, 0:1])
            rstd = row4("rstd")[0:2]
            recip(rstd, var)
            mr = row4("mr")[0:2]
            nc.vector.tensor_tensor(out=mr, in0=m2, in1=rstd, op=AL.mult)
            return tsc2, rstd, mr

        def apply_ln_gated(dst_big, pre_big, rAB, rBB, rCB, b):
            """dst = rAB*pre - rBB + rCB*seq (rCB None => init transform)."""
            for c in range(DC):
                t1 = work.tile([128, SP], F16, name="t1g", tag="gelu_tmp", bufs=2)
                nc.vector.tensor_tensor(out=t1, in0=rAB, in1=pre_big[:, c, :],
                                        op=AL.mult)
                if rCB is None:
                    nc.vector.tensor_tensor(out=dst_big[:, c, :], in0=t1,
                                            in1=rBB, op=AL.subtract)
                else:
                    nc.vector.tensor_tensor(out=t1, in0=t1, in1=rBB, op=AL.subtract)
                    t2 = work.tile([128, SP], F16, name="t2g", tag="gelu_tmp2")
                    tt(t2, rCB, seqT[b][:, c, :], AL.mult, gp=False)
                    nc.vector.tensor_tensor(out=dst_big[:, c, :], in0=t1, in1=t2,
                                            op=AL.add)

        # ================= initial transform (per group) ====================
        pre_t = []
        for g in range(2):
            ps6i = psum.tile([66, SP], F32, name=f"ps_init{g}", tag="ps6", bufs=2)
            for b in (2 * g, 2 * g + 1):
                sA = work_big(f"sA{b}", tag="axT", bufs=2)
                nc.sync.dma_start(out=sA,
                                  in_=seqT_in.ap()[b].rearrange("c p i -> p c i"))
                pre = work_big(f"pre{b}", tag="preT", bufs=4)
                for c in range(DC):
                    ps = psum.tile([128, SP], F32, name=f"ps_pre{b}{c}",
                                   tag="psmm", bufs=2)
                    mm(ps, [itW_t[k][:, c * 128:(c + 1) * 128] for k in range(2)],
                       [sA[:, 0, :], sA[:, 1, :]])
                    nc.scalar.activation(out=pre[:, c, :], in_=ps, func=AF.Copy)
                if b % 2 == 0:
                    nc.vector.memset(ps6i, 0.0)
                stat_mms(ps6i, b, pre, last=(b % 2 == 1))
                pre_t.append(pre)
            _, rstd, mr = ln_rows2(ps6i, want_tsc=False)
            rAi = row4(f"rAi{g}", F16)[0:2]
            nc.vector.tensor_tensor(out=rAi, in0=rstd, in1=mask2[g],
                                    op=AL.mult)
            nc.sync.dma_start(out=r2i_d[4 * g:4 * g + 2], in_=rAi)
            rBi = row4(f"rBi{g}", F16)[0:2]
            nc.vector.tensor_tensor(out=rBi, in0=mr, in1=mask2[g],
                                    op=AL.mult)
            nc.sync.dma_start(out=r2i_d[4 * g + 2:4 * g + 4], in_=rBi)
        for b in range(NB):
            g, gb = b // 2, b % 2
            rAB = bcast_read(r2i_d[4 * g + gb:4 * g + gb + 1, :], f"rAB0_{b}")
            rBB = bcast_read(r2i_d[4 * g + 2 + gb:4 * g + 3 + gb, :], f"rBB0_{b}")
            apply_ln_gated(seqT[b], pre_t[b], rAB, rBB, None, b)

        # ================= per-group row tail ===============================
        def emit_tail2(g, ps6, last):
            dsl = slice(2 * g, 2 * g + 2)
            tsc2, rstd, mr = ln_rows2(ps6)
            masked = row4("msk")[0:2]
            nc.vector.tensor_tensor(out=masked, in0=tsc2, in1=selp2[g],
                                    op=AL.mult)
            mx = tiny4("mx")[0:2]
            nc.vector.tensor_reduce(out=mx, in_=masked,
                                    axis=mybir.AxisListType.X, op=AL.max)
            negmx = tiny4("negmx")[0:2]
            nc.vector.tensor_scalar(out=negmx, in0=mx, scalar1=0.0,
                                    scalar2=-1.0, op0=AL.max, op1=AL.mult)
            et = row4("et")[0:2]
            nc.scalar.activation(out=et, in_=tsc2, func=AF.Exp, bias=negmx)
            nc.vector.tensor_tensor(out=et, in0=et, in1=selp2[g], op=AL.mult)
            en = tiny4("en")[0:2]
            nc.scalar.activation(out=en, in_=negmx, func=AF.Exp)
            nc.vector.tensor_scalar(out=en, in0=en, scalar1=EPS, scalar2=None,
                                    op0=AL.add)
            den = row4("den")[0:2]
            nc.vector.tensor_scalar(out=den, in0=et, scalar1=en, scalar2=None,
                                    op0=AL.add)
            dei = row4("dei")[0:2]
            recip(dei, den)
            tp = row4("tp")[0:2]
            nc.vector.tensor_tensor(out=tp, in0=et, in1=dei, op=AL.mult)

            # LN-apply rows -> r3_d[6g:6g+6] (rA {gb}, rB {2+gb}, rC {4+gb})
            tpm = row4("tpm")[0:2]
            nc.vector.tensor_tensor(out=tpm, in0=tp, in1=mask2[g], op=AL.mult)
            rAx = row4("rAx", F16)[0:2]
            nc.vector.tensor_tensor(out=rAx, in0=tpm, in1=rstd, op=AL.mult)
            nc.sync.dma_start(out=r3_d[6 * g:6 * g + 2], in_=rAx)
            rBx = row4("rBx", F16)[0:2]
            nc.vector.tensor_tensor(out=rBx, in0=tpm, in1=mr, op=AL.mult)
            nc.sync.dma_start(out=r3_d[6 * g + 2:6 * g + 4], in_=rBx)
            rCx = row4("rCx", F16)[0:2]
            nc.vector.tensor_tensor(out=rCx, in0=mask2[g], in1=tpm,
                                    op=AL.subtract)
            nc.sync.dma_start(out=r3_d[6 * g + 4:6 * g + 6], in_=rCx)

            if not last:
                tp16 = row4("tp16", F16)[0:2]
                nc.vector.tensor_copy(out=tp16, in_=tp)
                nc.sync.dma_start(out=ltp4_d[dsl], in_=tp16)
                # deact scan + active update
                nc.vector.tensor_scalar(out=nap2[g][:, 1:SP + 1], in0=a_row2[g],
                                        scalar1=-1.0, scalar2=1.0,
                                        op0=AL.mult, op1=AL.add)
                nc.vector.tensor_copy(out=tpp2[g][:, 1:SP + 1], in_=tp)
                u = row4("u")[0:2]
                nc.vector.tensor_tensor_scan(
                    out=u[:, ::-1], data0=nap2[g][:, SPP - 1:1:-1],
                    data1=tpp2[g][:, SPP - 1:1:-1], initial=0.0,
                    op0=AL.mult, op1=AL.add)
                nd = row4("nd")[0:2]
                nc.vector.tensor_tensor(out=nd, in0=a_row2[g], in1=u, op=AL.mult)
                nc.vector.tensor_scalar(out=nd, in0=nd, scalar1=-1.0, scalar2=1.0,
                                        op0=AL.mult, op1=AL.add)
                nc.vector.tensor_tensor(out=nd, in0=a_row2[g], in1=nd,
                                        op=AL.mult)
                nc.vector.tensor_scalar(out=nd, in0=nd, scalar1=0.0, scalar2=1.0,
                                        op0=AL.max, op1=AL.min)
                nc.vector.tensor_tensor(out=a_row2[g], in0=nd, in1=mask2[g],
                                        op=AL.mult)
                a16 = row4("a16", F16)[0:2]
                nc.vector.tensor_copy(out=a16, in_=a_row2[g])
                nc.sync.dma_start(out=a4_d[dsl], in_=a16)
                nar = row4("nar")[0:2]
                nc.vector.tensor_scalar(out=nar, in0=a_row2[g], scalar1=-1.0,
                                        scalar2=1.0, op0=AL.mult, op1=AL.add)
                nc.sync.dma_start(out=na4_d[dsl], in_=nar)

        # ================= main steps =======================================
        pending_tails = []
        for s in range(n_steps):
            ps6g = [psum.tile([66, SP], F32, name=f"ps6_{s}{g}", tag="ps6", bufs=2)
                    for g in range(2)]
            for b in range(NB):
                g, gb = b // 2, b % 2
                ps12 = ps6g[g]
                # ---- seq update from previous step's rows ----
                if s > 0:
                    rAB = bcast_read(r3_d[6 * g + gb:6 * g + gb + 1, :], f"rAB{b}")
                    rBB = bcast_read(r3_d[6 * g + 2 + gb:6 * g + 3 + gb, :],
                                     f"rBB{b}")
                    rCB = bcast_read(r3_d[6 * g + 4 + gb:6 * g + 5 + gb, :],
                                     f"rCB{b}")
                    apply_ln_gated(seqT[b], compS[b], rAB, rBB, rCB, b)

                # ---- phase A: broadcasts + base ----
                if s == 0:
                    aB = naB = None  # active == mask: scans are pure shifts
                else:
                    aB = bcast_read(a4_d[b:b + 1, :], f"aB{b}")
                    ltpB = bcast_read(ltp4_d[b:b + 1, :], f"ltpB{b}")
                    naB = work.tile([128, SPP], F32, name=f"naB{b}", tag="nabP",
                                    bufs=2)
                    nc.vector.memset(naB[:, 0:SPP:SPP - 1], 0.0)
                    nc.sync.dma_start(out=naB[:, 1:SP + 1],
                                      in_=_bcast_ap(na4_d[b:b + 1, :]))
                baseT = work_big(f"baseT{b}", tag="baseT", bufs=2)
                if s == 0:
                    for c in range(DC):
                        nc.vector.tensor_scalar(
                            out=baseT[:, c, :], in0=seqT[b][:, c, :],
                            scalar1=noc[:, c:c + 1], scalar2=None, op0=AL.add)
                else:
                    for c in range(DC):
                        tfc = work.tile([128, SP], F16, name=f"tfc{b}",
                                        tag="gate", bufs=3)
                        nc.vector.tensor_scalar(
                            out=tfc, in0=ltpB, scalar1=ymnc[:, c:c + 1],
                            scalar2=noc[:, c:c + 1], op0=AL.mult, op1=AL.add)
                        nc.vector.tensor_tensor(
                            out=baseT[:, c, :], in0=tfc, in1=seqT[b][:, c, :],
                            op=AL.add)

                # ---- phase B: 5 scans ----
                def fill_ax(axt, src_big, gp=True):
                    for c in range(DC):
                        nc.vector.memset(axt[:, c, 0:SPP:SPP - 1], 0.0)
                        tt(axt[:, c, 1:SP + 1], aB, src_big[:, c, :],
                           AL.mult, gp=True)

                def shift_copy(dst, src_big, sh):
                    """dst[i] = src[i-sh] (zeros shifted in); s==0 fast path
                    where every neighbor scan degenerates to a shift.  Pad
                    positions differ from the true recurrence but every
                    consumer there is masked (selp/mask zero)."""
                    for c in range(DC):
                        if sh > 0:
                            nc.vector.memset(dst[:, c, 0:sh], 0.0)
                            nc.vector.tensor_copy(out=dst[:, c, sh:SP],
                                                  in_=src_big[:, c, 0:SP - sh])
                        else:
                            nc.vector.memset(dst[:, c, SP + sh:SP], 0.0)
                            nc.vector.tensor_copy(out=dst[:, c, 0:SP + sh],
                                                  in_=src_big[:, c, -sh:SP])

                # lcT first: unblocks w1/w2 on PE while the l1/l2 chain runs
                lcT = work_big(f"lcT{b}", tag="lcT", bufs=2)
                if s == 0:
                    axB = None
                    shift_copy(lcT, seqT[b], 1)
                else:
                    axB = work.tile([128, DC, SPP], F16, name=f"axB{b}",
                                    tag="axT", bufs=2)
                    fill_ax(axB, seqT[b])
                    for c in range(DC):
                        scan_fwd(lcT[:, c, :], naB, axB[:, c])

                # deferred row tail of a completed group, one extra block
                # late so its bounce-DMA latency is fully hidden
                if gb == 1 and pending_tails:
                    emit_tail2(*pending_tails.pop(0))

                # w1 -> gelu -> interT issued early on PE
                interT = work.tile([128, 8, SP], FP8 if W2_FP8 else F16,
                                   name=f"interT{b}", tag="interT", bufs=2)
                if W1_FP8:
                    # fp8 copies on the (otherwise idle) Pool engine; lcT
                    # itself stays fp16 so the composer gating is unpolluted
                    lc8 = work_big(f"lc8{b}", tag="lc8", dtype=FP8, bufs=2)
                    seq8 = work_big(f"seq8{b}", tag="seq8", dtype=FP8, bufs=2)
                    for c in range(DC):
                        nc.gpsimd.tensor_tensor(out=lc8[:, c, :],
                                                in0=lcT[:, c, :], in1=ones16,
                                                op=AL.mult)
                        nc.gpsimd.tensor_tensor(out=seq8[:, c, :],
                                                in0=seqT[b][:, c, :], in1=ones16,
                                                op=AL.mult)
                    for hk in range(8):
                        ps = psum.tile([128, SP], F32, name=f"ps_w1{b}{hk}",
                                       tag="psmm", bufs=2)
                        mm_dr(ps, [w1W_t[0], w1W_t[1]], [lc8, seq8],
                              hk * 128, 128)
                        gelu_act(interT[:, hk, :], ps, scale=1.0 / WSCALE)
                else:
                    cc_rhs = [lcT[:, 0, :], lcT[:, 1, :],
                              seqT[b][:, 0, :], seqT[b][:, 1, :]]
                    for hk in range(8):
                        ps = psum.tile([128, SP], F32, name=f"ps_w1{b}{hk}",
                                       tag="psmm", bufs=2)
                        mm(ps, [w1W_t[k][:, hk * 128:(hk + 1) * 128]
                                for k in range(4)], cc_rhs)
                        gelu_act(interT[:, hk, :], ps)

                l1T = work_big(f"l1T{b}", tag="l1T", bufs=2)
                r1T = work_big(f"r1T{b}", tag="r1T", bufs=2)
                l2T = work_big(f"l2T{b}", tag="l2T", bufs=2)
                r2T = work_big(f"r2T{b}", tag="r2T", bufs=2)
                if s == 0:
                    shift_copy(l1T, baseT, 1)
                    shift_copy(r1T, baseT, -1)
                    shift_copy(l2T, baseT, 2)
                    shift_copy(r2T, baseT, -2)
                else:
                    fill_ax(axB, baseT)
                    for c in range(DC):
                        scan_fwd(l1T[:, c, :], naB, axB[:, c])
                        scan_bwd(r1T[:, c, :], naB, axB[:, c])
                    ax2 = work.tile([128, DC, SPP], F16, name=f"ax2{b}",
                                    tag="axT", bufs=2)
                    fill_ax(ax2, l1T)
                    for c in range(DC):
                        scan_fwd(l2T[:, c, :], naB, ax2[:, c])
                    fill_ax(ax2, r1T)
                    for c in range(DC):
                        scan_bwd(r2T[:, c, :], naB, ax2[:, c])

                # ---- phase C: conv (transposed) + score ----
                piece_order = [(2, baseT), (1, l1T), (3, r1T), (0, l2T), (4, r2T)]
                gT = work_big(f"gT{b}", tag="gpar", bufs=2)
                for c in range(DC):
                    ps = psum.tile([128, SP], F32, name=f"ps_cv{b}{c}", tag="psmm", bufs=2)
                    lhsT, rhs = [], []
                    for w, piece in piece_order:
                        for ci in range(DC):
                            lhsT.append(convW_t[w * DC + ci][:, c * 128:(c + 1) * 128])
                            rhs.append(piece[:, ci, :])
                    mm(ps, lhsT, rhs)
                    gelu_act(gT[:, c, :], ps)
                # score -> stats psum rows 64-65 via one-hot scW columns
                if gb == 0:
                    nc.vector.memset(ps12, 0.0)
                mm_acc(ps12, [scw6_t[b][:, c, :] for c in range(DC)],
                       [gT[:, c, :] for c in range(DC)])

                # ---- phase E: w2 -> gated sum ----
                comp = compS[b]
                parT = work_big(f"parT{b}", tag="gpar", bufs=2)
                inter_lhsT = [interT[:, hk, :] for hk in range(8)]
                for gg in [3, 0, 1, 2]:
                    for c in range(DC):
                        cc = gg * DC + c
                        ps = psum.tile([128, SP], F32, name=f"ps_w2{b}{cc}",
                                       tag="psmm", bufs=2)
                        if W2_FP8:
                            mm_dr(ps, w2W_t, [interT[:, 2 * p:2 * p + 2, :]
                                              for p in range(4)], cc * 128, 128)
                            osc = 1.0 / WSCALE
                        else:
                            mm(ps, [w2W_t[hk][:, cc * 128:(cc + 1) * 128]
                                    for hk in range(8)], inter_lhsT)
                            osc = 1.0
                        if gg == 3:
                            nc.scalar.activation(out=parT[:, c, :], in_=ps,
                                                 func=AF.Copy, scale=osc)
                        else:
                            gate = work.tile([128, SP], F16, name=f"gate{b}",
                                             tag="gate", bufs=3)
                            nc.scalar.activation(out=gate, in_=ps, func=AF.Sigmoid,
                                                 bias=0.0, scale=osc)
                            src = [lcT, seqT[b], parT][gg]
                            if gg == 0:
                                nc.vector.tensor_tensor(out=comp[:, c, :], in0=gate,
                                                        in1=src[:, c, :],
                                                        op=AL.mult)
                            else:
                                gm = work.tile([128, SP], F16, name=f"gm{b}",
                                               tag="gelu_tmp2")
                                tt(gm, gate, src[:, c, :], AL.mult, gp=False)
                                nc.vector.tensor_tensor(out=comp[:, c, :],
                                                        in0=comp[:, c, :],
                                                        in1=gm, op=AL.add)
                # LN stats for comp -> stats psum rows 0-1 / 32-33
                stat_mms(ps12, b, comp, last=(gb == 1))
                if gb == 1:
                    pending_tails.append((g, ps12, s == n_steps - 1))

        while pending_tails:
            emit_tail2(*pending_tails.pop(0))

        # final seq update
        for b in range(NB):
            g, gb = b // 2, b % 2
            rAB = bcast_read(r3_d[6 * g + gb:6 * g + gb + 1, :], f"rABf{b}")
            rBB = bcast_read(r3_d[6 * g + 2 + gb:6 * g + 3 + gb, :], f"rBBf{b}")
            rCB = bcast_read(r3_d[6 * g + 4 + gb:6 * g + 5 + gb, :], f"rCBf{b}")
            apply_ln_gated(seqT[b], compS[b], rAB, rBB, rCB, b)

        # ---------------- output ------------------------------------------
        for b in range(NB):
            for c in range(DC):
                nc.sync.dma_start(out=out_dram.ap()[b, c],
                                  in_=seqT[b][:, c, 0:S2])
    return nc


def _host_prep(inputs):
    f32 = np.float32
    f16 = np.float16
    seq = np.asarray(inputs["sequence"], f32)
    im = np.asarray(inputs["input_mask"], f32)
    START = np.asarray(inputs["START"], f32)
    END = np.asarray(inputs["END"], f32)
    yes_t = np.asarray(inputs["yes_t"], f32).reshape(-1)
    no_t = np.asarray(inputs["no_t"], f32).reshape(-1)
    N, S, Dd = seq.shape
    assert (N, S, Dd) == (32, 512, 256), (N, S, Dd)

    ones = np.ones((N, 1, 1), f32)
    zeros = np.zeros((N, 1, 1), f32)
    mask0 = np.concatenate([ones, im], 1)
    mask_no_end = np.concatenate([mask0, zeros], 1)
    mask_yes_end = np.concatenate([ones, mask0], 1)
    END_mask = mask_yes_end - mask_no_end
    seqA = np.concatenate([np.broadcast_to(START, (N, 1, Dd)), seq,
                           np.zeros((N, 1, Dd), f32)], 1)
    seqA = (END_mask * END + (1.0 - END_mask) * seqA).astype(f32)
    mask = mask_yes_end
    mask_no_start = np.concatenate([zeros, mask[:, 1:]], 1)
    last_tok = np.concatenate([END_mask[:, 1:], zeros], 1)
    selp = (mask_no_start * mask_no_end * (1.0 - last_tok)).astype(f32)

    seqT = np.zeros((N, DC, 128, SP), f16)
    for c in range(DC):
        seqT[:, c, :, :S2] = seqA[:, :, c * 128:(c + 1) * 128].transpose(0, 2, 1)
    maskP = np.zeros((N, SP), f32)
    maskP[:, :S2] = mask[:, :, 0]
    selpP = np.zeros((N, SP), f32)
    selpP[:, :S2] = selp[:, :, 0]

    def chunk_col(v, nch):
        return np.ascontiguousarray(np.asarray(v, f32).reshape(nch, 128).T)

    scWc = chunk_col(np.asarray(inputs["scW"], f32).reshape(-1), DC)
    scw6 = np.zeros((NB, 128, DC, 66), f16)
    for b in range(NB):
        for c in range(DC):
            scw6[b, :, c, 64 + b % 2] = scWc[:, c]

    host = {
        "seqT": seqT, "mask": maskP, "selp": selpP,
        "a16": maskP.astype(f16), "na32": (1.0 - maskP).astype(f32),
        "itW": np.asarray(inputs["itW"], f16),
        "convW": np.asarray(inputs["convW"], f16),
        "scw6": scw6,
        "noc": chunk_col(no_t, DC),
        "ymnc": chunk_col(yes_t - no_t, DC),
    }

    def fp8_pairs(w, npairs, M):
        import ml_dtypes
        w = np.asarray(w, f32) * WSCALE
        out = np.zeros((npairs, 128, 2, M), np.uint8)
        for p in range(npairs):
            for j in range(2):
                blk = w[(2 * p + j) * 128:(2 * p + j + 1) * 128, :]
                out[p, :, j, :] = blk.astype(ml_dtypes.float8_e4m3fn).view(np.uint8)
        return out

    if W1_FP8:
        host["w1p"] = fp8_pairs(inputs["w1W"], 2, H)
    else:
        host["w1W"] = np.asarray(inputs["w1W"], f16)
    if W2_FP8:
        host["w2p"] = fp8_pairs(inputs["w2W"], 4, 4 * D)
    else:
        host["w2W"] = np.asarray(inputs["w2W"], f16)
    return host


_PROG_CACHE = {}


def kernel(**inputs):
    global LAST_EXEC_NS, LAST_RES, LAST_NC
    n_steps = int(inputs["n_steps"])
    host = _host_prep(inputs)

    key = (n_steps, SIM, GP_LVL, W1_FP8, W2_FP8)
    if key not in _PROG_CACHE:
        _PROG_CACHE[key] = _build_program(n_steps)
    nc = _PROG_CACHE[key]
    LAST_NC = nc

    per_batch = {"seqT", "mask", "selp", "a16", "na32"}
    in_maps = []
    for k in range(NCORES):
        m = {}
        for name, arr in host.items():
            if name in per_batch:
                m[name] = np.ascontiguousarray(arr[k * NB:(k + 1) * NB])
            else:
                m[name] = arr
        in_maps.append(m)

    if SIM:
        from concourse.bass_interp import CoreSim
        results = []
        for k in range(int(os.environ.get("CRVNN_SIM_CORES", "1"))):
            sim = CoreSim(nc)
            for name, v in in_maps[k].items():
                sim.tensor(name)[:] = v
            sim.simulate()
            results.append(np.array(sim.tensor("out")))
    else:
        from concourse.bass_utils import run_bass_kernel_spmd
        if not getattr(nc, "_waitfix_done", False):
            _split_multiwaits(nc)
            nc._waitfix_done = True
        res = run_bass_kernel_spmd(nc, in_maps, list(range(NCORES)), trace=TRACE)
        LAST_EXEC_NS = res.exec_time_ns
        LAST_RES = res
        results = [res.results[k]["out"] for k in range(NCORES)]

    full = np.zeros((32, S2, D), np.float32)
    for k, o in enumerate(results):
        for b in range(NB):
            for c in range(DC):
                full[k * NB + b, :, c * 128:(c + 1) * 128] = \
                    o[b, c].astype(np.float32).T
    return full


# revision 44
# speedup vs baseline: 1.1462x; 1.1462x over previous
"""CRvNN forward kernel for 8x Trainium2 NeuronCores (Bass/Tile).

Strategy
--------
Pure data parallelism: batch 32 -> 4 per core; params replicated; no
collectives.  On-device state lives in TRANSPOSED layout (D=256 on partitions
as 2x128 chunks, sequence position i on the free axis, padded 514 -> 516).

The reference's (S2 x S2) neighbor-probability matrices are first-order
linear recurrences; each (lnp @ x) / (rnp @ x) is ONE DVE tensor_tensor_scan
per 128-partition chunk.  No S^2 matrices, no PE transposes.

v2: fp16 datapath (TensorTensor 2x / tensor_scalar 4x DVE modes, halved DMA,
1 cyc/row matmuls).  The scan coefficient na=1-a stays fp32 (fp16 rounding
would compound over up-to-514 factors); row math stays fp32.

v3: batched row pipeline.  A (1,516) DVE op costs the same as (128,516), so
all per-batch row math runs once per step on (4,SP)/(12,SP) tiles.  Per-batch
LN/score sums land in a shared (12,SP) PSUM via one-hot lhsT columns
(m rows 0-3, v rows 4-7, tsc rows 8-11), copied out by a single ACT op with a
per-partition scale column.  Bounce-buffer DMA writes drop to 4 per step.

This walrus build supports only ONE sync wait per instruction; a
post-scheduling pass splits multi-wait instructions into single-wait NOP
chains.
"""
import os
import sys
from contextlib import ExitStack

import numpy as np

sys.path.insert(0, "/opt/trn_rl_repo")

import bass_rust
import concourse.bass as bass
import concourse.mybir as mybir
from concourse.tile import TileContext

F32 = mybir.dt.float32
F16 = mybir.dt.float16
U8 = mybir.dt.uint8
FP8 = mybir.dt.float8e4
AL = mybir.AluOpType
AF = mybir.ActivationFunctionType
PM = mybir.MatmulPerfMode

NCORES = 8
NB = 4            # batch per core
D = 256
DC = 2            # D chunks of 128
S2 = 514
SP = 516          # padded sequence length
SPP = SP + 2      # scan-input tiles have leading+trailing zero pad columns
H = 1024
WIN = 5
EPS = 1e-9

SIM = os.environ.get("CRVNN_SIM", "0") == "1"
TRACE = os.environ.get("CRVNN_TRACE", "0") == "1"
GP_LVL = int(os.environ.get("CRVNN_GP", "0"))
W1_FP8 = os.environ.get("CRVNN_W1", "fp8") == "fp8"
W2_FP8 = os.environ.get("CRVNN_W2", "fp8") == "fp8"
WSCALE = 64.0

NSPLITS = [(0, 512), (512, SP - 512)]

LAST_EXEC_NS = None
LAST_RES = None
LAST_NC = None


# --------------------------------------------------------------------------
# post-scheduling fixup: split multi-wait instructions into 1-wait NOP chains
# --------------------------------------------------------------------------
def _split_multiwaits(nc):
    counter = [0]

    def mk_nop(engine, wait):
        counter[0] += 1
        n = bass_rust.InstNoOp(name=f"WFIX-{counter[0]}", ins=[], outs=[])
        n.engine = engine
        n.sync_info = bass_rust.SyncInfo(on_wait=[wait], on_update=[])
        return n

    total = 0
    for f in nc.m.functions:
        for bb in f.blocks:
            out = []
            changed = False
            for inst in list(bb.instructions):
                si = inst.sync_info
                waits = list(si.on_wait) if (si is not None and si.on_wait) else []
                if len(waits) > 1:
                    for w in waits[:-1]:
                        out.append(mk_nop(inst.engine, w))
                    inst.sync_info = bass_rust.SyncInfo(
                        on_wait=[waits[-1]],
                        on_update=list(si.on_update) if si.on_update else [])
                    changed = True
                    total += 1
                out.append(inst)
            if changed:
                bb.instructions = out
    return total


def _bcast_ap(drow):
    """DRAM row AP (1, n) -> partition-broadcast AP (128, n)."""
    return bass.AP(tensor=drow.tensor, offset=drow.offset,
                   ap=[[0, 128]] + drow.ap[1:])


def _build_program(n_steps):
    nc = bass.Bass()

    seqT_in = nc.declare_dram_parameter("seqT", [NB, DC, 128, SP], F16, isOutput=False)
    mask_in = nc.declare_dram_parameter("mask", [NB, SP], F32, isOutput=False)
    selp_in = nc.declare_dram_parameter("selp", [NB, SP], F32, isOutput=False)
    a16_in = nc.declare_dram_parameter("a16", [NB, SP], F16, isOutput=False)
    na32_in = nc.declare_dram_parameter("na32", [NB, SP], F32, isOutput=False)
    itW_in = nc.declare_dram_parameter("itW", [D, D], F16, isOutput=False)
    convW_in = nc.declare_dram_parameter("convW", [WIN * D, D], F16, isOutput=False)
    scw6_in = nc.declare_dram_parameter("scw6", [NB, 128, DC, 66], F16,
                                        isOutput=False)
    if W1_FP8:
        w1W_in = nc.declare_dram_parameter("w1p", [2, 128, 2, H], U8,
                                           isOutput=False)
    else:
        w1W_in = nc.declare_dram_parameter("w1W", [2 * D, H], F16, isOutput=False)
    if W2_FP8:
        w2W_in = nc.declare_dram_parameter("w2p", [4, 128, 2, 4 * D], U8,
                                           isOutput=False)
    else:
        w2W_in = nc.declare_dram_parameter("w2W", [H, 4 * D], F16, isOutput=False)
    noc_in = nc.declare_dram_parameter("noc", [128, DC], F32, isOutput=False)
    ymn_in = nc.declare_dram_parameter("ymnc", [128, DC], F32, isOutput=False)
    out_dram = nc.declare_dram_parameter("out", [NB, DC, 128, S2], F16, isOutput=True)

    with TileContext(nc) as tc, ExitStack() as ctx:
        wpool = ctx.enter_context(tc.tile_pool(name="wpool", bufs=1))
        state = ctx.enter_context(tc.tile_pool(name="state", bufs=1))
        work = ctx.enter_context(tc.tile_pool(name="work", bufs=1))
        psum = ctx.enter_context(tc.tile_pool(name="psum", bufs=1, space="PSUM"))
        dram = ctx.enter_context(tc.tile_pool(name="dramp", bufs=1, space="DRAM"))

        # ---------------- weights -> SBUF (fp16, direct DMA) ----------------
        def load_w(name, dram_ap, shape, q=None):
            t = wpool.tile(shape, F16, name=name)
            (q or nc.scalar).dma_start(out=t, in_=dram_ap)
            return t

        convW_t = [load_w(f"convW{k}", convW_in.ap()[k * 128:(k + 1) * 128, :],
                          [128, D]) for k in range(10)]
        def load_u8(name, dram_ap, shape):
            t = wpool.tile(shape, U8, name=name)
            nc.scalar.dma_start(out=t, in_=dram_ap)
            return t

        if W1_FP8:
            w1W_t = [load_u8(f"w1p{p}", w1W_in.ap()[p], [128, 2, H])
                     for p in range(2)]
        else:
            w1W_t = [load_w(f"w1W{k}", w1W_in.ap()[k * 128:(k + 1) * 128, :],
                            [128, H]) for k in range(4)]
        if W2_FP8:
            w2W_t = [load_u8(f"w2p{p}", w2W_in.ap()[p], [128, 2, H])
                     for p in range(4)]
        else:
            w2W_t = [load_w(f"w2W{k}", w2W_in.ap()[k * 128:(k + 1) * 128, :],
                            [128, H]) for k in range(8)]
        scw6_t = [load_w(f"scw6_{b}", scw6_in.ap()[b], [128, DC, 66])
                  for b in range(NB)]
        itW_t = [load_w(f"itW{k}", itW_in.ap()[k * 128:(k + 1) * 128, :],
                        [128, D], q=nc.sync) for k in range(2)]

        noc = wpool.tile([128, DC], F32)
        nc.sync.dma_start(out=noc, in_=noc_in.ap())
        ymnc = wpool.tile([128, DC], F32)
        nc.sync.dma_start(out=ymnc, in_=ymn_in.ap())
        eps_t = wpool.tile([128, 1], F32)
        nc.vector.memset(eps_t, 1e-5)
        ones16 = wpool.tile([128, SP], F16)
        nc.vector.memset(ones16, 1.0)

        # one-hot stat lhsT columns (group-local, 32-aligned stat rows):
        # onesm[b] col b%2 = 1 (mean), onesv[b] col 32+b%2 = 1 (meansq);
        # score lhsT (scw66) has col 64+b%2 = scW.  Engine SBUF/PSUM accesses
        # must start at 32-aligned partitions, hence the spread layout.
        onesm, onesv = [], []
        for b in range(NB):
            gb = b % 2
            tm = wpool.tile([128, 66], F16, name=f"onesm{b}")
            nc.vector.memset(tm, 0.0)
            nc.vector.memset(tm[:, gb:gb + 1], 1.0)
            onesm.append(tm)
            tv = wpool.tile([128, 66], F16, name=f"onesv{b}")
            nc.vector.memset(tv, 0.0)
            nc.vector.memset(tv[:, 32 + gb:33 + gb], 1.0)
            onesv.append(tv)


        # ---------------- persistent state ----------------------------------
        seqT = [state.tile([128, DC, SP], F16, name=f"seqT{b}") for b in range(NB)]
        compS = [state.tile([128, DC, SP], F16, name=f"compS{b}") for b in range(NB)]
        a_row2 = [state.tile([2, SP], F32, name=f"a_row2{g}") for g in range(2)]
        mask2 = [state.tile([2, SP], F32, name=f"mask2{g}") for g in range(2)]
        selp2 = [state.tile([2, SP], F32, name=f"selp2{g}") for g in range(2)]
        for g in range(2):
            nc.sync.dma_start(out=a_row2[g], in_=mask_in.ap()[2 * g:2 * g + 2])
            nc.sync.dma_start(out=mask2[g], in_=mask_in.ap()[2 * g:2 * g + 2])
            nc.sync.dma_start(out=selp2[g], in_=selp_in.ap()[2 * g:2 * g + 2])
        nap2 = [state.tile([2, SPP], F32, name=f"nap2{g}") for g in range(2)]
        tpp2 = [state.tile([2, SPP], F32, name=f"tpp2{g}") for g in range(2)]
        for g in range(2):
            nc.vector.memset(nap2[g][:, 0:SPP:SPP - 1], 0.0)
            nc.vector.memset(tpp2[g][:, 0:SPP:SPP - 1], 0.0)
        # persistent scan-input tiles: pad columns zeroed once, data columns
        # overwritten每 step (fills write [:, c, 1:SP+1] only)
        axS = [state.tile([128, DC, SPP], F16, name=f"axS{b}") for b in range(NB)]
        ax2S = [state.tile([128, DC, SPP], F16, name=f"ax2S{b}") for b in range(NB)]
        naBS = [state.tile([128, SPP], F32, name=f"naBS{b}") for b in range(NB)]
        for b in range(NB):
            for c in range(DC):
                nc.vector.memset(axS[b][:, c, 0:SPP:SPP - 1], 0.0)
                nc.vector.memset(ax2S[b][:, c, 0:SPP:SPP - 1], 0.0)
            nc.vector.memset(naBS[b][:, 0:SPP:SPP - 1], 0.0)

        # DRAM bounce tiles for partition-broadcast
        a4_d = dram.tile([4, SP], F16, name="a4_d")
        na4_d = dram.tile([4, SP], F32, name="na4_d")
        ltp4_d = dram.tile([4, SP], F16, name="ltp4_d")
        r3_d = dram.tile([12, SP], F16, name="r3_d")
        r2i_d = dram.tile([8, SP], F16, name="r2i_d")

        def work_big(name, tag, dtype=F16, bufs=None):
            return work.tile([128, DC, SP], dtype, name=name, tag=tag, bufs=bufs)

        def row4(name, dtype=F32):
            return work.tile([4, SP], dtype, name=name, tag="rowW", bufs=10)

        def tiny4(name):
            return work.tile([4, 1], F32, name=name, tag="tinyW", bufs=6)

        def bc_tile(name, dtype=F16):
            return work.tile([128, SP], dtype, name=name, tag="bcast", bufs=12)

        def bcast_read(drow_ap, name):
            t = bc_tile(name)
            nc.sync.dma_start(out=t, in_=_bcast_ap(drow_ap))
            return t

        def recip(out_r, in_r):
            nc.vector.reciprocal(out=out_r, in_=in_r)

        def tt(out, in0, in1, op, gp=False):
            eng = nc.gpsimd if (gp and GP_LVL > 0) else nc.vector
            eng.tensor_tensor(out=out, in0=in0, in1=in1, op=op)

        def mm_dr(psum_ap, wpairs, rhs_pairs, col0, ncols):
            """fp8 DoubleRow: contract pairs of 128-k-chunks per instruction.
            wpairs: uint8 [128,2,M] tiles; rhs_pairs: fp8 [128,2,SP] APs."""
            P = len(wpairs)
            for (o, s) in NSPLITS:
                for p in range(P):
                    nc.tensor.matmul(
                        psum_ap[:, o:o + s],
                        wpairs[p].bitcast(FP8)[:, :, col0:col0 + ncols],
                        rhs_pairs[p][:, :, o:o + s],
                        start=(p == 0), stop=(p == P - 1),
                        perf_mode=PM.DoubleRow)

        def mm(psum_ap, lhsT, rhs_chunks, nsl=NSPLITS):
            K = len(lhsT)
            for (o, s) in nsl:
                for k in range(K):
                    nc.tensor.matmul(psum_ap[:, o:o + s], lhsT[k],
                                     rhs_chunks[k][:, o:o + s],
                                     start=(k == 0), stop=(k == K - 1))

        def gelu_act(out, in_, scale=1.0):
            if SIM:
                x2 = work.tile([out.shape[0], out.shape[-1]], F32, name="gx2",
                               tag="gelu_tmp", bufs=2)
                nc.scalar.activation(out=x2, in_=in_, func=AF.Square, bias=0.0,
                                     scale=scale)
                nc.vector.tensor_scalar(out=x2, in0=x2, scalar1=0.044715,
                                        scalar2=1.0, op0=AL.mult, op1=AL.add)
                u = work.tile([out.shape[0], out.shape[-1]], F32, name="gu",
                              tag="gelu_tmp2")
                nc.scalar.activation(out=u, in_=in_, func=AF.Copy, scale=scale)
                nc.vector.tensor_tensor(out=x2, in0=x2, in1=u, op=AL.mult)
                nc.scalar.activation(out=x2, in_=x2, func=AF.Tanh,
                                     scale=0.7978845608028654)
                nc.vector.tensor_scalar(out=x2, in0=x2, scalar1=1.0,
                                        scalar2=0.5, op0=AL.add, op1=AL.mult)
                nc.vector.tensor_tensor(out=out, in0=x2, in1=u, op=AL.mult)
            else:
                nc.scalar.activation(out=out, in_=in_, func=AF.Gelu_apprx_tanh,
                                     bias=0.0, scale=scale)

        def scan_fwd(out_c, nap, datap):
            """out[i] = data[i-1] + na[i-1]*out[i-1]; data pad supplies z0=0."""
            nc.vector.tensor_tensor_scan(
                out=out_c, data0=nap[:, 0:SP], data1=datap[:, 0:SP],
                initial=0.0, op0=AL.mult, op1=AL.add)

        def scan_bwd(out_c, nap, datap):
            nc.vector.tensor_tensor_scan(
                out=out_c[:, ::-1], data0=nap[:, SPP - 1:1:-1],
                data1=datap[:, SPP - 1:1:-1], initial=0.0,
                op0=AL.mult, op1=AL.add)

        def mm_acc(psum_ap, lhsT, rhs_chunks, first=False, last=False):
            """Matmuls into a shared accumulation group: only the very first
            call (per split region) zeroes PSUM, only the last closes it."""
            K = len(lhsT)
            for (o, s) in NSPLITS:
                for k in range(K):
                    nc.tensor.matmul(psum_ap[:, o:o + s], lhsT[k],
                                     rhs_chunks[k][:, o:o + s],
                                     start=(first and k == 0),
                                     stop=(last and k == K - 1),
                                     skip_group_check=True)

        def stat_mms(ps6, b, src_big, last=False):
            """Accumulate batch b's LN sums into its group's stats psum."""
            mm_acc(ps6, [onesm[b], onesm[b]],
                   [src_big[:, 0, :], src_big[:, 1, :]])
            sq = [work.tile([128, SP], F16, name=f"sq{c}", tag="sq", bufs=2)
                  for c in range(DC)]
            for c in range(DC):
                nc.scalar.activation(out=sq[c], in_=src_big[:, c, :],
                                     func=AF.Square, bias=0.0)
            mm_acc(ps6, [onesv[b], onesv[b]], [sq[0], sq[1]], last=last)

        def ln_rows2(ps6, want_tsc=True):
            """(66,SP) psum (m 0-1, v 32-33, tsc 64-65) -> base-0 rows."""
            m2 = row4("m2")[0:2]
            v2 = row4("v2")[0:2]
            tsc2 = row4("tsc2")[0:2] if want_tsc else None
            for (o, s) in NSPLITS:
                nc.scalar.activation(out=m2[:, o:o + s], in_=ps6[0:2, o:o + s],
                                     func=AF.Copy, scale=1.0 / D)
                nc.scalar.activation(out=v2[:, o:o + s], in_=ps6[32:34, o:o + s],
                                     func=AF.Copy, scale=1.0 / D)
                if want_tsc:
                    nc.scalar.activation(out=tsc2[:, o:o + s],
                                         in_=ps6[64:66, o:o + s], func=AF.Copy)
            msq = row4("msq")[0:2]
            nc.vector.tensor_tensor(out=msq, in0=m2, in1=m2, op=AL.mult)
            var = row4("var")[0:2]
            nc.vector.tensor_tensor(out=var, in0=v2, in1=msq, op=AL.subtract)
            nc.scalar.activation(out=var, in_=var, func=AF.Sqrt,
                                 bias=eps_t[0:2, 0:1])
            rstd = row4("rstd")[0:2]
            recip(rstd, var)
            mr = row4("mr")[0:2]
            nc.vector.tensor_tensor(out=mr, in0=m2, in1=rstd, op=AL.mult)
            return tsc2, rstd, mr

        def apply_ln_gated(dst_big, pre_big, rAB, rBB, rCB, b):
            """dst = rAB*pre - rBB + rCB*seq (rCB None => init transform)."""
            for c in range(DC):
                t1 = work.tile([128, SP], F16, name="t1g", tag="gelu_tmp", bufs=2)
                nc.vector.tensor_tensor(out=t1, in0=rAB, in1=pre_big[:, c, :],
                                        op=AL.mult)
                if rCB is None:
                    nc.vector.tensor_tensor(out=dst_big[:, c, :], in0=t1,
                                            in1=rBB, op=AL.subtract)
                else:
                    nc.vector.tensor_tensor(out=t1, in0=t1, in1=rBB, op=AL.subtract)
                    t2 = work.tile([128, SP], F16, name="t2g", tag="gelu_tmp2")
                    tt(t2, rCB, seqT[b][:, c, :], AL.mult, gp=False)
                    nc.vector.tensor_tensor(out=dst_big[:, c, :], in0=t1, in1=t2,
                                            op=AL.add)

        # ================= initial transform (per group) ====================
        pre_t = []
        for g in range(2):
            ps6i = psum.tile([66, SP], F32, name=f"ps_init{g}", tag="ps6", bufs=2)
            for b in (2 * g, 2 * g + 1):
                sA = work_big(f"sA{b}", tag="axT", bufs=2)
                nc.sync.dma_start(out=sA,
                                  in_=seqT_in.ap()[b].rearrange("c p i -> p c i"))
                pre = work_big(f"pre{b}", tag="preT", bufs=4)
                for c in range(DC):
                    ps = psum.tile([128, SP], F32, name=f"ps_pre{b}{c}",
                                   tag="psmm", bufs=2)
                    mm(ps, [itW_t[k][:, c * 128:(c + 1) * 128] for k in range(2)],
                       [sA[:, 0, :], sA[:, 1, :]])
                    nc.scalar.activation(out=pre[:, c, :], in_=ps, func=AF.Copy)
                if b % 2 == 0:
                    nc.vector.memset(ps6i, 0.0)
                stat_mms(ps6i, b, pre, last=(b % 2 == 1))
                pre_t.append(pre)
            _, rstd, mr = ln_rows2(ps6i, want_tsc=False)
            rAi = row4(f"rAi{g}", F16)[0:2]
            nc.vector.tensor_tensor(out=rAi, in0=rstd, in1=mask2[g],
                                    op=AL.mult)
            nc.sync.dma_start(out=r2i_d[4 * g:4 * g + 2], in_=rAi)
            rBi = row4(f"rBi{g}", F16)[0:2]
            nc.vector.tensor_tensor(out=rBi, in0=mr, in1=mask2[g],
                                    op=AL.mult)
            nc.sync.dma_start(out=r2i_d[4 * g + 2:4 * g + 4], in_=rBi)
        for b in range(NB):
            g, gb = b // 2, b % 2
            rAB = bcast_read(r2i_d[4 * g + gb:4 * g + gb + 1, :], f"rAB0_{b}")
            rBB = bcast_read(r2i_d[4 * g + 2 + gb:4 * g + 3 + gb, :], f"rBB0_{b}")
            apply_ln_gated(seqT[b], pre_t[b], rAB, rBB, None, b)

        # ================= per-group row tail ===============================
        def emit_tail2(g, ps6, last):
            dsl = slice(2 * g, 2 * g + 2)
            tsc2, rstd, mr = ln_rows2(ps6)
            masked = row4("msk")[0:2]
            nc.vector.tensor_tensor(out=masked, in0=tsc2, in1=selp2[g],
                                    op=AL.mult)
            mx = tiny4("mx")[0:2]
            nc.vector.tensor_reduce(out=mx, in_=masked,
                                    axis=mybir.AxisListType.X, op=AL.max)
            negmx = tiny4("negmx")[0:2]
            nc.vector.tensor_scalar(out=negmx, in0=mx, scalar1=0.0,
                                    scalar2=-1.0, op0=AL.max, op1=AL.mult)
            et = row4("et")[0:2]
            nc.scalar.activation(out=et, in_=tsc2, func=AF.Exp, bias=negmx)
            nc.vector.tensor_tensor(out=et, in0=et, in1=selp2[g], op=AL.mult)
            en = tiny4("en")[0:2]
            nc.scalar.activation(out=en, in_=negmx, func=AF.Exp)
            nc.vector.tensor_scalar(out=en, in0=en, scalar1=EPS, scalar2=None,
                                    op0=AL.add)
            den = row4("den")[0:2]
            nc.vector.tensor_scalar(out=den, in0=et, scalar1=en, scalar2=None,
                                    op0=AL.add)
            dei = row4("dei")[0:2]
            recip(dei, den)
            tp = row4("tp")[0:2]
            nc.vector.tensor_tensor(out=tp, in0=et, in1=dei, op=AL.mult)

            # LN-apply rows -> r3_d[6g:6g+6] (rA {gb}, rB {2+gb}, rC {4+gb})
            tpm = row4("tpm")[0:2]
            nc.vector.tensor_tensor(out=tpm, in0=tp, in1=mask2[g], op=AL.mult)
            rAx = row4("rAx", F16)[0:2]
            nc.vector.tensor_tensor(out=rAx, in0=tpm, in1=rstd, op=AL.mult)
            nc.sync.dma_start(out=r3_d[6 * g:6 * g + 2], in_=rAx)
            rBx = row4("rBx", F16)[0:2]
            nc.vector.tensor_tensor(out=rBx, in0=tpm, in1=mr, op=AL.mult)
            nc.sync.dma_start(out=r3_d[6 * g + 2:6 * g + 4], in_=rBx)
            rCx = row4("rCx", F16)[0:2]
            nc.vector.tensor_tensor(out=rCx, in0=mask2[g], in1=tpm,
                                    op=AL.subtract)
            nc.sync.dma_start(out=r3_d[6 * g + 4:6 * g + 6], in_=rCx)

            if not last:
                tp16 = row4("tp16", F16)[0:2]
                nc.vector.tensor_copy(out=tp16, in_=tp)
                nc.sync.dma_start(out=ltp4_d[dsl], in_=tp16)
                # deact scan + active update
                nc.vector.tensor_scalar(out=nap2[g][:, 1:SP + 1], in0=a_row2[g],
                                        scalar1=-1.0, scalar2=1.0,
                                        op0=AL.mult, op1=AL.add)
                nc.vector.tensor_copy(out=tpp2[g][:, 1:SP + 1], in_=tp)
                u = row4("u")[0:2]
                nc.vector.tensor_tensor_scan(
                    out=u[:, ::-1], data0=nap2[g][:, SPP - 1:1:-1],
                    data1=tpp2[g][:, SPP - 1:1:-1], initial=0.0,
                    op0=AL.mult, op1=AL.add)
                nd = row4("nd")[0:2]
                nc.vector.tensor_tensor(out=nd, in0=a_row2[g], in1=u, op=AL.mult)
                nc.vector.tensor_scalar(out=nd, in0=nd, scalar1=-1.0, scalar2=1.0,
                                        op0=AL.mult, op1=AL.add)
                nc.vector.tensor_tensor(out=nd, in0=a_row2[g], in1=nd,
                                        op=AL.mult)
                nc.vector.tensor_scalar(out=nd, in0=nd, scalar1=0.0, scalar2=1.0,
                                        op0=AL.max, op1=AL.min)
                nc.vector.tensor_tensor(out=a_row2[g], in0=nd, in1=mask2[g],
                                        op=AL.mult)
                a16 = row4("a16", F16)[0:2]
                nc.vector.tensor_copy(out=a16, in_=a_row2[g])
                nc.sync.dma_start(out=a4_d[dsl], in_=a16)
                nar = row4("nar")[0:2]
                nc.vector.tensor_scalar(out=nar, in0=a_row2[g], scalar1=-1.0,
                                        scalar2=1.0, op0=AL.mult, op1=AL.add)
                nc.sync.dma_start(out=na4_d[dsl], in_=nar)

        # ================= deferred per-batch back half =====================
        # w2 -> gated sum -> LN stats.  Emitted one batch late so the DVE
        # gating never waits on the PE/ACT w1->gelu->w2->sigmoid pipeline.
        def emit_back(b, ps12, lcT, interT, g, gb, s):
            comp = compS[b]
            parT = work_big(f"parT{b}", tag="gpar", bufs=3)
            inter_lhsT = [interT[:, hk, :] for hk in range(8)]
            for gg in [3, 0, 1, 2]:
                for c in range(DC):
                    cc = gg * DC + c
                    ps = psum.tile([128, SP], F32, name=f"ps_w2{b}{cc}",
                                   tag="psmm", bufs=2)
                    if W2_FP8:
                        mm_dr(ps, w2W_t, [interT[:, 2 * p:2 * p + 2, :]
                                          for p in range(4)], cc * 128, 128)
                        osc = 1.0 / WSCALE
                    else:
                        mm(ps, [w2W_t[hk][:, cc * 128:(cc + 1) * 128]
                                for hk in range(8)], inter_lhsT)
                        osc = 1.0
                    if gg == 3:
                        nc.scalar.activation(out=parT[:, c, :], in_=ps,
                                             func=AF.Copy, scale=osc)
                    else:
                        gate = work.tile([128, SP], F16, name=f"gate{b}",
                                         tag="gate", bufs=4)
                        nc.scalar.activation(out=gate, in_=ps, func=AF.Sigmoid,
                                             bias=0.0, scale=osc)
                        srcx = [lcT, seqT[b], parT][gg]
                        if gg == 0:
                            nc.vector.tensor_tensor(out=comp[:, c, :], in0=gate,
                                                    in1=srcx[:, c, :],
                                                    op=AL.mult)
                        else:
                            gm = work.tile([128, SP], F16, name=f"gm{b}",
                                           tag="gelu_tmp2")
                            tt(gm, gate, srcx[:, c, :], AL.mult, gp=True)
                            nc.vector.tensor_tensor(out=comp[:, c, :],
                                                    in0=comp[:, c, :],
                                                    in1=gm, op=AL.add)
            stat_mms(ps12, b, comp, last=(gb == 1))
            if gb == 1:
                pending_tails.append((g, ps12, s == n_steps - 1))

        # ================= main steps =======================================
        pending_tails = []
        pending_back = []
        for s in range(n_steps):
            ps6g = [psum.tile([66, SP], F32, name=f"ps6_{s}{g}", tag="ps6", bufs=2)
                    for g in range(2)]
            for b in range(NB):
                g, gb = b // 2, b % 2
                ps12 = ps6g[g]
                # ---- seq update from previous step's rows ----
                if s > 0:
                    rAB = bcast_read(r3_d[6 * g + gb:6 * g + gb + 1, :], f"rAB{b}")
                    rBB = bcast_read(r3_d[6 * g + 2 + gb:6 * g + 3 + gb, :],
                                     f"rBB{b}")
                    rCB = bcast_read(r3_d[6 * g + 4 + gb:6 * g + 5 + gb, :],
                                     f"rCB{b}")
                    apply_ln_gated(seqT[b], compS[b], rAB, rBB, rCB, b)

                # ---- phase A: broadcasts + base ----
                if s == 0:
                    aB = naB = None  # active == mask: scans are pure shifts
                else:
                    aB = bcast_read(a4_d[b:b + 1, :], f"aB{b}")
                    ltpB = bcast_read(ltp4_d[b:b + 1, :], f"ltpB{b}")
                    naB = naBS[b]
                    nc.sync.dma_start(out=naB[:, 1:SP + 1],
                                      in_=_bcast_ap(na4_d[b:b + 1, :]))
                baseT = work_big(f"baseT{b}", tag="baseT", bufs=2)
                if s == 0:
                    for c in range(DC):
                        nc.vector.tensor_scalar(
                            out=baseT[:, c, :], in0=seqT[b][:, c, :],
                            scalar1=noc[:, c:c + 1], scalar2=None, op0=AL.add)
                else:
                    for c in range(DC):
                        tfc = work.tile([128, SP], F16, name=f"tfc{b}",
                                        tag="gate", bufs=4)
                        nc.vector.tensor_scalar(
                            out=tfc, in0=ltpB, scalar1=ymnc[:, c:c + 1],
                            scalar2=noc[:, c:c + 1], op0=AL.mult, op1=AL.add)
                        nc.vector.tensor_tensor(
                            out=baseT[:, c, :], in0=tfc, in1=seqT[b][:, c, :],
                            op=AL.add)

                # ---- phase B: 5 scans ----
                def fill_ax(axt, src_big):
                    for c in range(DC):
                        tt(axt[:, c, 1:SP + 1], aB, src_big[:, c, :],
                           AL.mult, gp=(GP_LVL >= 2))

                def shift_copy(dst, src_big, sh):
                    """dst[i] = src[i-sh] (zeros shifted in); s==0 fast path
                    where every neighbor scan degenerates to a shift.  Pad
                    positions differ from the true recurrence but every
                    consumer there is masked (selp/mask zero)."""
                    for c in range(DC):
                        if sh > 0:
                            nc.vector.memset(dst[:, c, 0:sh], 0.0)
                            nc.vector.tensor_copy(out=dst[:, c, sh:SP],
                                                  in_=src_big[:, c, 0:SP - sh])
                        else:
                            nc.vector.memset(dst[:, c, SP + sh:SP], 0.0)
                            nc.vector.tensor_copy(out=dst[:, c, 0:SP + sh],
                                                  in_=src_big[:, c, -sh:SP])

                # lcT first: unblocks w1/w2 on PE while the l1/l2 chain runs
                lcT = work_big(f"lcT{b}", tag="lcT", bufs=2)
                if s == 0:
                    axB = None
                    shift_copy(lcT, seqT[b], 1)
                else:
                    axB = axS[b]
                    fill_ax(axB, seqT[b])
                    for c in range(DC):
                        scan_fwd(lcT[:, c, :], naB, axB[:, c])

                # deferred row tail of a completed group, one extra block
                # late so its bounce-DMA latency is fully hidden
                if gb == 1 and pending_tails:
                    emit_tail2(*pending_tails.pop(0))

                # previous batch's w2/gating: its gates are ready by now, and
                # its PE work stays directly behind that batch's score
                if pending_back:
                    emit_back(*pending_back.pop(0))

                # w1 -> gelu -> interT issued early on PE
                interT = work.tile([128, 8, SP], FP8 if W2_FP8 else F16,
                                   name=f"interT{b}", tag="interT", bufs=2)
                if W1_FP8:
                    # fp8 copies on the (otherwise idle) Pool engine; lcT
                    # itself stays fp16 so the composer gating is unpolluted
                    lc8 = work_big(f"lc8{b}", tag="lc8", dtype=FP8, bufs=2)
                    seq8 = work_big(f"seq8{b}", tag="seq8", dtype=FP8, bufs=2)
                    for c in range(DC):
                        nc.gpsimd.tensor_tensor(out=lc8[:, c, :],
                                                in0=lcT[:, c, :], in1=ones16,
                                                op=AL.mult)
                        nc.gpsimd.tensor_tensor(out=seq8[:, c, :],
                                                in0=seqT[b][:, c, :], in1=ones16,
                                                op=AL.mult)
                    for hk in range(8):
                        ps = psum.tile([128, SP], F32, name=f"ps_w1{b}{hk}",
                                       tag="psmm", bufs=2)
                        mm_dr(ps, [w1W_t[0], w1W_t[1]], [lc8, seq8],
                              hk * 128, 128)
                        gelu_act(interT[:, hk, :], ps, scale=1.0 / WSCALE)
                else:
                    cc_rhs = [lcT[:, 0, :], lcT[:, 1, :],
                              seqT[b][:, 0, :], seqT[b][:, 1, :]]
                    for hk in range(8):
                        ps = psum.tile([128, SP], F32, name=f"ps_w1{b}{hk}",
                                       tag="psmm", bufs=2)
                        mm(ps, [w1W_t[k][:, hk * 128:(hk + 1) * 128]
                                for k in range(4)], cc_rhs)
                        gelu_act(interT[:, hk, :], ps)

                l1T = work_big(f"l1T{b}", tag="l1T", bufs=2)
                r1T = work_big(f"r1T{b}", tag="r1T", bufs=2)
                l2T = work_big(f"l2T{b}", tag="l2T", bufs=2)
                r2T = work_big(f"r2T{b}", tag="r2T", bufs=2)
                if s == 0:
                    shift_copy(l1T, baseT, 1)
                    shift_copy(r1T, baseT, -1)
                    shift_copy(l2T, baseT, 2)
                    shift_copy(r2T, baseT, -2)
                else:
                    fill_ax(axB, baseT)
                    for c in range(DC):
                        scan_fwd(l1T[:, c, :], naB, axB[:, c])
                        scan_bwd(r1T[:, c, :], naB, axB[:, c])
                    ax2 = ax2S[b]
                    fill_ax(ax2, l1T)
                    for c in range(DC):
                        scan_fwd(l2T[:, c, :], naB, ax2[:, c])
                    fill_ax(ax2, r1T)
                    for c in range(DC):
                        scan_bwd(r2T[:, c, :], naB, ax2[:, c])

                # ---- phase C: conv (transposed) + score ----
                piece_order = [(2, baseT), (1, l1T), (3, r1T), (0, l2T), (4, r2T)]
                gT = work_big(f"gT{b}", tag="gpar", bufs=3)
                for c in range(DC):
                    ps = psum.tile([128, SP], F32, name=f"ps_cv{b}{c}", tag="psmm", bufs=2)
                    lhsT, rhs = [], []
                    for w, piece in piece_order:
                        for ci in range(DC):
                            lhsT.append(convW_t[w * DC + ci][:, c * 128:(c + 1) * 128])
                            rhs.append(piece[:, ci, :])
                    mm(ps, lhsT, rhs)
                    gelu_act(gT[:, c, :], ps)
                # score -> stats psum rows 64-65 via one-hot scW columns
                if gb == 0:
                    nc.vector.memset(ps12, 0.0)
                mm_acc(ps12, [scw6_t[b][:, c, :] for c in range(DC)],
                       [gT[:, c, :] for c in range(DC)])

                pending_back.append((b, ps12, lcT, interT, g, gb, s))

        while pending_back:
            emit_back(*pending_back.pop(0))
        while pending_tails:
            emit_tail2(*pending_tails.pop(0))

        # final seq update
        for b in range(NB):
            g, gb = b // 2, b % 2
            rAB = bcast_read(r3_d[6 * g + gb:6 * g + gb + 1, :], f"rABf{b}")
            rBB = bcast_read(r3_d[6 * g + 2 + gb:6 * g + 3 + gb, :], f"rBBf{b}")
            rCB = bcast_read(r3_d[6 * g + 4 + gb:6 * g + 5 + gb, :], f"rCBf{b}")
            apply_ln_gated(seqT[b], compS[b], rAB, rBB, rCB, b)

        # ---------------- output ------------------------------------------
        for b in range(NB):
            for c in range(DC):
                nc.sync.dma_start(out=out_dram.ap()[b, c],
                                  in_=seqT[b][:, c, 0:S2])
    return nc


def _host_prep(inputs):
    f32 = np.float32
    f16 = np.float16
    seq = np.asarray(inputs["sequence"], f32)
    im = np.asarray(inputs["input_mask"], f32)
    START = np.asarray(inputs["START"], f32)
    END = np.asarray(inputs["END"], f32)
    yes_t = np.asarray(inputs["yes_t"], f32).reshape(-1)
    no_t = np.asarray(inputs["no_t"], f32).reshape(-1)
    N, S, Dd = seq.shape
    assert (N, S, Dd) == (32, 512, 256), (N, S, Dd)

    ones = np.ones((N, 1, 1), f32)
    zeros = np.zeros((N, 1, 1), f32)
    mask0 = np.concatenate([ones, im], 1)
    mask_no_end = np.concatenate([mask0, zeros], 1)
    mask_yes_end = np.concatenate([ones, mask0], 1)
    END_mask = mask_yes_end - mask_no_end
    seqA = np.concatenate([np.broadcast_to(START, (N, 1, Dd)), seq,
                           np.zeros((N, 1, Dd), f32)], 1)
    seqA = (END_mask * END + (1.0 - END_mask) * seqA).astype(f32)
    mask = mask_yes_end
    mask_no_start = np.concatenate([zeros, mask[:, 1:]], 1)
    last_tok = np.concatenate([END_mask[:, 1:], zeros], 1)
    selp = (mask_no_start * mask_no_end * (1.0 - last_tok)).astype(f32)

    seqT = np.zeros((N, DC, 128, SP), f16)
    for c in range(DC):
        seqT[:, c, :, :S2] = seqA[:, :, c * 128:(c + 1) * 128].transpose(0, 2, 1)
    maskP = np.zeros((N, SP), f32)
    maskP[:, :S2] = mask[:, :, 0]
    selpP = np.zeros((N, SP), f32)
    selpP[:, :S2] = selp[:, :, 0]

    def chunk_col(v, nch):
        return np.ascontiguousarray(np.asarray(v, f32).reshape(nch, 128).T)

    scWc = chunk_col(np.asarray(inputs["scW"], f32).reshape(-1), DC)
    scw6 = np.zeros((NB, 128, DC, 66), f16)
    for b in range(NB):
        for c in range(DC):
            scw6[b, :, c, 64 + b % 2] = scWc[:, c]

    host = {
        "seqT": seqT, "mask": maskP, "selp": selpP,
        "a16": maskP.astype(f16), "na32": (1.0 - maskP).astype(f32),
        "itW": np.asarray(inputs["itW"], f16),
        "convW": np.asarray(inputs["convW"], f16),
        "scw6": scw6,
        "noc": chunk_col(no_t, DC),
        "ymnc": chunk_col(yes_t - no_t, DC),
    }

    def fp8_pairs(w, npairs, M):
        import ml_dtypes
        w = np.asarray(w, f32) * WSCALE
        out = np.zeros((npairs, 128, 2, M), np.uint8)
        for p in range(npairs):
            for j in range(2):
                blk = w[(2 * p + j) * 128:(2 * p + j + 1) * 128, :]
                out[p, :, j, :] = blk.astype(ml_dtypes.float8_e4m3fn).view(np.uint8)
        return out

    if W1_FP8:
        host["w1p"] = fp8_pairs(inputs["w1W"], 2, H)
    else:
        host["w1W"] = np.asarray(inputs["w1W"], f16)
    if W2_FP8:
        host["w2p"] = fp8_pairs(inputs["w2W"], 4, 4 * D)
    else:
        host["w2W"] = np.asarray(inputs["w2W"], f16)
    return host


_PROG_CACHE = {}


def kernel(**inputs):
    global LAST_EXEC_NS, LAST_RES, LAST_NC
    n_steps = int(inputs["n_steps"])
    host = _host_prep(inputs)

    key = (n_steps, SIM, GP_LVL, W1_FP8, W2_FP8)
    if key not in _PROG_CACHE:
        _PROG_CACHE[key] = _build_program(n_steps)
    nc = _PROG_CACHE[key]
    LAST_NC = nc

    per_batch = {"seqT", "mask", "selp", "a16", "na32"}
    in_maps = []
    for k in range(NCORES):
        m = {}
        for name, arr in host.items():
            if name in per_batch:
                m[name] = np.ascontiguousarray(arr[k * NB:(k + 1) * NB])
            else:
                m[name] = arr
        in_maps.append(m)

    if SIM:
        from concourse.bass_interp import CoreSim
        results = []
        for k in range(int(os.environ.get("CRVNN_SIM_CORES", "1"))):
            sim = CoreSim(nc)
            for name, v in in_maps[k].items():
                sim.tensor(name)[:] = v
            sim.simulate()
            results.append(np.array(sim.tensor("out")))
    else:
        from concourse.bass_utils import run_bass_kernel_spmd
        if not getattr(nc, "_waitfix_done", False):
            _split_multiwaits(nc)
            nc._waitfix_done = True
        res = run_bass_kernel_spmd(nc, in_maps, list(range(NCORES)), trace=TRACE)
        LAST_EXEC_NS = res.exec_time_ns
        LAST_RES = res
        results = [res.results[k]["out"] for k in range(NCORES)]

    full = np.zeros((32, S2, D), np.float32)
    for k, o in enumerate(results):
        for b in range(NB):
            for c in range(DC):
                full[k * NB + b, :, c * 128:(c + 1) * 128] = \
                    o[b, c].astype(np.float32).T
    return full


# revision 46
# speedup vs baseline: 1.1996x; 1.0465x over previous
"""CRvNN forward kernel for 8x Trainium2 NeuronCores (Bass/Tile).

Strategy
--------
Pure data parallelism: batch 32 -> 4 per core; params replicated; no
collectives.  On-device state lives in TRANSPOSED layout (D=256 on partitions
as 2x128 chunks, sequence position i on the free axis, padded 514 -> 516).

The reference's (S2 x S2) neighbor-probability matrices are first-order
linear recurrences; each (lnp @ x) / (rnp @ x) is ONE DVE tensor_tensor_scan
per 128-partition chunk.  No S^2 matrices, no PE transposes.

v2: fp16 datapath (TensorTensor 2x / tensor_scalar 4x DVE modes, halved DMA,
1 cyc/row matmuls).  The scan coefficient na=1-a stays fp32 (fp16 rounding
would compound over up-to-514 factors); row math stays fp32.

v3: batched row pipeline.  A (1,516) DVE op costs the same as (128,516), so
all per-batch row math runs once per step on (4,SP)/(12,SP) tiles.  Per-batch
LN/score sums land in a shared (12,SP) PSUM via one-hot lhsT columns
(m rows 0-3, v rows 4-7, tsc rows 8-11), copied out by a single ACT op with a
per-partition scale column.  Bounce-buffer DMA writes drop to 4 per step.

This walrus build supports only ONE sync wait per instruction; a
post-scheduling pass splits multi-wait instructions into single-wait NOP
chains.
"""
import os
import sys
from contextlib import ExitStack

import numpy as np

sys.path.insert(0, "/opt/trn_rl_repo")

import bass_rust
import concourse.bass as bass
import concourse.mybir as mybir
from concourse.tile import TileContext

F32 = mybir.dt.float32
F16 = mybir.dt.float16
U8 = mybir.dt.uint8
FP8 = mybir.dt.float8e4
AL = mybir.AluOpType
AF = mybir.ActivationFunctionType
PM = mybir.MatmulPerfMode

NCORES = 8
NB = 4            # batch per core
D = 256
DC = 2            # D chunks of 128
S2 = 514
SP = 516          # padded sequence length
SPP = SP + 2      # scan-input tiles have leading+trailing zero pad columns
H = 1024
WIN = 5
EPS = 1e-9

SIM = os.environ.get("CRVNN_SIM", "0") == "1"
TRACE = os.environ.get("CRVNN_TRACE", "0") == "1"
GP_LVL = int(os.environ.get("CRVNN_GP", "0"))
W1_FP8 = os.environ.get("CRVNN_W1", "fp8") == "fp8"
W2_FP8 = os.environ.get("CRVNN_W2", "fp8") == "fp8"
WSCALE = 64.0

NSPLITS = [(0, 512), (512, SP - 512)]

LAST_EXEC_NS = None
LAST_RES = None
LAST_NC = None


# --------------------------------------------------------------------------
# post-scheduling fixup: split multi-wait instructions into 1-wait NOP chains
# --------------------------------------------------------------------------
def _split_multiwaits(nc):
    counter = [0]

    def mk_nop(engine, wait):
        counter[0] += 1
        n = bass_rust.InstNoOp(name=f"WFIX-{counter[0]}", ins=[], outs=[])
        n.engine = engine
        n.sync_info = bass_rust.SyncInfo(on_wait=[wait], on_update=[])
        return n

    total = 0
    for f in nc.m.functions:
        for bb in f.blocks:
            out = []
            changed = False
            for inst in list(bb.instructions):
                si = inst.sync_info
                waits = list(si.on_wait) if (si is not None and si.on_wait) else []
                if len(waits) > 1:
                    for w in waits[:-1]:
                        out.append(mk_nop(inst.engine, w))
                    inst.sync_info = bass_rust.SyncInfo(
                        on_wait=[waits[-1]],
                        on_update=list(si.on_update) if si.on_update else [])
                    changed = True
                    total += 1
                out.append(inst)
            if changed:
                bb.instructions = out
    return total


def _bcast_ap(drow):
    """DRAM row AP (1, n) -> partition-broadcast AP (128, n)."""
    return bass.AP(tensor=drow.tensor, offset=drow.offset,
                   ap=[[0, 128]] + drow.ap[1:])


def _build_program(n_steps):
    nc = bass.Bass()

    seqT_in = nc.declare_dram_parameter("seqT", [NB, DC, 128, SP], F16, isOutput=False)
    mask_in = nc.declare_dram_parameter("mask", [NB, SP], F32, isOutput=False)
    selp_in = nc.declare_dram_parameter("selp", [NB, SP], F32, isOutput=False)
    a16_in = nc.declare_dram_parameter("a16", [NB, SP], F16, isOutput=False)
    na32_in = nc.declare_dram_parameter("na32", [NB, SP], F32, isOutput=False)
    itW_in = nc.declare_dram_parameter("itW", [D, D], F16, isOutput=False)
    convW_in = nc.declare_dram_parameter("convW", [WIN * D, D], F16, isOutput=False)
    scw6_in = nc.declare_dram_parameter("scw6", [NB, 128, DC, 66], F16,
                                        isOutput=False)
    if W1_FP8:
        w1W_in = nc.declare_dram_parameter("w1p", [2, 128, 2, H], U8,
                                           isOutput=False)
    else:
        w1W_in = nc.declare_dram_parameter("w1W", [2 * D, H], F16, isOutput=False)
    if W2_FP8:
        w2W_in = nc.declare_dram_parameter("w2p", [4, 128, 2, 4 * D], U8,
                                           isOutput=False)
    else:
        w2W_in = nc.declare_dram_parameter("w2W", [H, 4 * D], F16, isOutput=False)
    noc_in = nc.declare_dram_parameter("noc", [128, DC], F32, isOutput=False)
    ymn_in = nc.declare_dram_parameter("ymnc", [128, DC], F32, isOutput=False)
    out_dram = nc.declare_dram_parameter("out", [NB, DC, 128, S2], F16, isOutput=True)

    with TileContext(nc) as tc, ExitStack() as ctx:
        wpool = ctx.enter_context(tc.tile_pool(name="wpool", bufs=1))
        state = ctx.enter_context(tc.tile_pool(name="state", bufs=1))
        work = ctx.enter_context(tc.tile_pool(name="work", bufs=1))
        psum = ctx.enter_context(tc.tile_pool(name="psum", bufs=1, space="PSUM"))
        dram = ctx.enter_context(tc.tile_pool(name="dramp", bufs=1, space="DRAM"))

        # ---------------- weights -> SBUF (fp16, direct DMA) ----------------
        def load_w(name, dram_ap, shape, q=None):
            t = wpool.tile(shape, F16, name=name)
            (q or nc.scalar).dma_start(out=t, in_=dram_ap)
            return t

        convW_t = [load_w(f"convW{k}", convW_in.ap()[k * 128:(k + 1) * 128, :],
                          [128, D]) for k in range(10)]
        def load_u8(name, dram_ap, shape):
            t = wpool.tile(shape, U8, name=name)
            nc.scalar.dma_start(out=t, in_=dram_ap)
            return t

        if W1_FP8:
            w1W_t = [load_u8(f"w1p{p}", w1W_in.ap()[p], [128, 2, H])
                     for p in range(2)]
        else:
            w1W_t = [load_w(f"w1W{k}", w1W_in.ap()[k * 128:(k + 1) * 128, :],
                            [128, H]) for k in range(4)]
        if W2_FP8:
            w2W_t = [load_u8(f"w2p{p}", w2W_in.ap()[p], [128, 2, H])
                     for p in range(4)]
        else:
            w2W_t = [load_w(f"w2W{k}", w2W_in.ap()[k * 128:(k + 1) * 128, :],
                            [128, H]) for k in range(8)]
        scw6_t = [load_w(f"scw6_{b}", scw6_in.ap()[b], [128, DC, 66])
                  for b in range(NB)]
        itW_t = [load_w(f"itW{k}", itW_in.ap()[k * 128:(k + 1) * 128, :],
                        [128, D], q=nc.sync) for k in range(2)]

        noc = wpool.tile([128, DC], F32)
        nc.sync.dma_start(out=noc, in_=noc_in.ap())
        ymnc = wpool.tile([128, DC], F32)
        nc.sync.dma_start(out=ymnc, in_=ymn_in.ap())
        eps_t = wpool.tile([128, 1], F32)
        nc.vector.memset(eps_t, 1e-5)
        ones16 = wpool.tile([128, SP], F16)
        nc.vector.memset(ones16, 1.0)

        # one-hot stat lhsT columns (group-local, 32-aligned stat rows):
        # onesm[b] col b%2 = 1 (mean), onesv[b] col 32+b%2 = 1 (meansq);
        # score lhsT (scw66) has col 64+b%2 = scW.  Engine SBUF/PSUM accesses
        # must start at 32-aligned partitions, hence the spread layout.
        onesm, onesv = [], []
        for b in range(NB):
            gb = b % 2
            tm = wpool.tile([128, 66], F16, name=f"onesm{b}")
            nc.vector.memset(tm, 0.0)
            nc.vector.memset(tm[:, gb:gb + 1], 1.0)
            onesm.append(tm)
            tv = wpool.tile([128, 66], F16, name=f"onesv{b}")
            nc.vector.memset(tv, 0.0)
            nc.vector.memset(tv[:, 32 + gb:33 + gb], 1.0)
            onesv.append(tv)


        # ---------------- persistent state ----------------------------------
        seqT = [state.tile([128, DC, SP], F16, name=f"seqT{b}") for b in range(NB)]
        compS = [state.tile([128, DC, SP], F16, name=f"compS{b}") for b in range(NB)]
        a_row2 = [state.tile([2, SP], F32, name=f"a_row2{g}") for g in range(2)]
        mask2 = [state.tile([2, SP], F32, name=f"mask2{g}") for g in range(2)]
        selp2 = [state.tile([2, SP], F32, name=f"selp2{g}") for g in range(2)]
        for g in range(2):
            nc.sync.dma_start(out=a_row2[g], in_=mask_in.ap()[2 * g:2 * g + 2])
            nc.sync.dma_start(out=mask2[g], in_=mask_in.ap()[2 * g:2 * g + 2])
            nc.sync.dma_start(out=selp2[g], in_=selp_in.ap()[2 * g:2 * g + 2])
        mask16 = [state.tile([2, SP], F16, name=f"mask16{g}") for g in range(2)]
        selp16 = [state.tile([2, SP], F16, name=f"selp16{g}") for g in range(2)]
        for g in range(2):
            nc.vector.tensor_copy(out=mask16[g], in_=mask2[g])
            nc.vector.tensor_copy(out=selp16[g], in_=selp2[g])
        nap2 = [state.tile([2, SPP], F32, name=f"nap2{g}") for g in range(2)]
        tpp2 = [state.tile([2, SPP], F16, name=f"tpp2{g}") for g in range(2)]
        for g in range(2):
            nc.vector.memset(nap2[g][:, 0:SPP:SPP - 1], 0.0)
            nc.vector.memset(tpp2[g][:, 0:SPP:SPP - 1], 0.0)
        # persistent scan-input tiles: pad columns zeroed once, data columns
        # overwritten每 step (fills write [:, c, 1:SP+1] only)
        axS = [state.tile([128, DC, SPP], F16, name=f"axS{b}") for b in range(NB)]
        ax2S = [state.tile([128, DC, SPP], F16, name=f"ax2S{b}") for b in range(NB)]
        naBS = [state.tile([128, SPP], F32, name=f"naBS{b}") for b in range(NB)]
        for b in range(NB):
            for c in range(DC):
                nc.vector.memset(axS[b][:, c, 0:SPP:SPP - 1], 0.0)
                nc.vector.memset(ax2S[b][:, c, 0:SPP:SPP - 1], 0.0)
            nc.vector.memset(naBS[b][:, 0:SPP:SPP - 1], 0.0)

        # DRAM bounce tiles for partition-broadcast
        a4_d = dram.tile([4, SP], F16, name="a4_d")
        na4_d = dram.tile([4, SP], F32, name="na4_d")
        ltp4_d = dram.tile([4, SP], F16, name="ltp4_d")
        r3_d = dram.tile([12, SP], F16, name="r3_d")
        r2i_d = dram.tile([8, SP], F16, name="r2i_d")

        def work_big(name, tag, dtype=F16, bufs=None):
            return work.tile([128, DC, SP], dtype, name=name, tag=tag, bufs=bufs)

        def row4(name, dtype=F32):
            return work.tile([4, SP], dtype, name=name, tag="rowW", bufs=10)

        def tiny4(name):
            return work.tile([4, 1], F32, name=name, tag="tinyW", bufs=6)

        def bc_tile(name, dtype=F16):
            return work.tile([128, SP], dtype, name=name, tag="bcast", bufs=12)

        def bcast_read(drow_ap, name):
            t = bc_tile(name)
            nc.sync.dma_start(out=t, in_=_bcast_ap(drow_ap))
            return t

        def recip(out_r, in_r):
            with nc.allow_low_precision(reason="fp16 rstd/dei rows: 5e-4 rel "
                                        "is far inside the 2e-2 budget"):
                nc.vector.reciprocal(out=out_r, in_=in_r)

        def tt(out, in0, in1, op, gp=False):
            eng = nc.gpsimd if (gp and GP_LVL > 0) else nc.vector
            eng.tensor_tensor(out=out, in0=in0, in1=in1, op=op)

        def mm_dr(psum_ap, wpairs, rhs_pairs, col0, ncols):
            """fp8 DoubleRow: contract pairs of 128-k-chunks per instruction.
            wpairs: uint8 [128,2,M] tiles; rhs_pairs: fp8 [128,2,SP] APs."""
            P = len(wpairs)
            for (o, s) in NSPLITS:
                for p in range(P):
                    nc.tensor.matmul(
                        psum_ap[:, o:o + s],
                        wpairs[p].bitcast(FP8)[:, :, col0:col0 + ncols],
                        rhs_pairs[p][:, :, o:o + s],
                        start=(p == 0), stop=(p == P - 1),
                        perf_mode=PM.DoubleRow)

        def mm(psum_ap, lhsT, rhs_chunks, nsl=NSPLITS):
            K = len(lhsT)
            for (o, s) in nsl:
                for k in range(K):
                    nc.tensor.matmul(psum_ap[:, o:o + s], lhsT[k],
                                     rhs_chunks[k][:, o:o + s],
                                     start=(k == 0), stop=(k == K - 1))

        def gelu_act(out, in_, scale=1.0):
            if SIM:
                x2 = work.tile([out.shape[0], out.shape[-1]], F32, name="gx2",
                               tag="gelu_tmp", bufs=2)
                nc.scalar.activation(out=x2, in_=in_, func=AF.Square, bias=0.0,
                                     scale=scale)
                nc.vector.tensor_scalar(out=x2, in0=x2, scalar1=0.044715,
                                        scalar2=1.0, op0=AL.mult, op1=AL.add)
                u = work.tile([out.shape[0], out.shape[-1]], F32, name="gu",
                              tag="gelu_tmp2")
                nc.scalar.activation(out=u, in_=in_, func=AF.Copy, scale=scale)
                nc.vector.tensor_tensor(out=x2, in0=x2, in1=u, op=AL.mult)
                nc.scalar.activation(out=x2, in_=x2, func=AF.Tanh,
                                     scale=0.7978845608028654)
                nc.vector.tensor_scalar(out=x2, in0=x2, scalar1=1.0,
                                        scalar2=0.5, op0=AL.add, op1=AL.mult)
                nc.vector.tensor_tensor(out=out, in0=x2, in1=u, op=AL.mult)
            else:
                nc.scalar.activation(out=out, in_=in_, func=AF.Gelu_apprx_tanh,
                                     bias=0.0, scale=scale)

        def scan_fwd(out_c, nap, datap):
            """out[i] = data[i-1] + na[i-1]*out[i-1]; data pad supplies z0=0."""
            nc.vector.tensor_tensor_scan(
                out=out_c, data0=nap[:, 0:SP], data1=datap[:, 0:SP],
                initial=0.0, op0=AL.mult, op1=AL.add)

        def scan_bwd(out_c, nap, datap):
            nc.vector.tensor_tensor_scan(
                out=out_c[:, ::-1], data0=nap[:, SPP - 1:1:-1],
                data1=datap[:, SPP - 1:1:-1], initial=0.0,
                op0=AL.mult, op1=AL.add)

        def mm_acc(psum_ap, lhsT, rhs_chunks, first=False, last=False):
            """Matmuls into a shared accumulation group: only the very first
            call (per split region) zeroes PSUM, only the last closes it."""
            K = len(lhsT)
            for (o, s) in NSPLITS:
                for k in range(K):
                    nc.tensor.matmul(psum_ap[:, o:o + s], lhsT[k],
                                     rhs_chunks[k][:, o:o + s],
                                     start=(first and k == 0),
                                     stop=(last and k == K - 1),
                                     skip_group_check=True)

        def stat_mms(ps6, b, src_big, last=False):
            """Accumulate batch b's LN sums into its group's stats psum."""
            mm_acc(ps6, [onesm[b], onesm[b]],
                   [src_big[:, 0, :], src_big[:, 1, :]])
            sq = [work.tile([128, SP], F16, name=f"sq{c}", tag="sq", bufs=2)
                  for c in range(DC)]
            for c in range(DC):
                nc.scalar.activation(out=sq[c], in_=src_big[:, c, :],
                                     func=AF.Square, bias=0.0)
            mm_acc(ps6, [onesv[b], onesv[b]], [sq[0], sq[1]], last=last)

        def ln_rows2(ps6, want_tsc=True):
            """(66,SP) psum (m 0-1, v 32-33, tsc 64-65) -> base-0 rows."""
            m2 = row4("m2")[0:2]
            v2 = row4("v2")[0:2]
            tsc2 = row4("tsc2", F16)[0:2] if want_tsc else None
            for (o, s) in NSPLITS:
                nc.scalar.activation(out=m2[:, o:o + s], in_=ps6[0:2, o:o + s],
                                     func=AF.Copy, scale=1.0 / D)
                nc.scalar.activation(out=v2[:, o:o + s], in_=ps6[32:34, o:o + s],
                                     func=AF.Copy, scale=1.0 / D)
                if want_tsc:
                    nc.scalar.activation(out=tsc2[:, o:o + s],
                                         in_=ps6[64:66, o:o + s], func=AF.Copy)
            msq = row4("msq")[0:2]
            nc.vector.tensor_tensor(out=msq, in0=m2, in1=m2, op=AL.mult)
            var = row4("var")[0:2]
            nc.vector.tensor_tensor(out=var, in0=v2, in1=msq, op=AL.subtract)
            nc.scalar.activation(out=var, in_=var, func=AF.Sqrt,
                                 bias=eps_t[0:2, 0:1])
            rstd = row4("rstd", F16)[0:2]
            recip(rstd, var)
            mr = row4("mr", F16)[0:2]
            nc.vector.tensor_tensor(out=mr, in0=m2, in1=rstd, op=AL.mult)
            return tsc2, rstd, mr

        def apply_ln_gated(dst_big, pre_big, rAB, rBB, rCB, b):
            """dst = rAB*pre - rBB + rCB*seq (rCB None => init transform)."""
            for c in range(DC):
                t1 = work.tile([128, SP], F16, name="t1g", tag="gelu_tmp", bufs=2)
                nc.vector.tensor_tensor(out=t1, in0=rAB, in1=pre_big[:, c, :],
                                        op=AL.mult)
                if rCB is None:
                    nc.vector.tensor_tensor(out=dst_big[:, c, :], in0=t1,
                                            in1=rBB, op=AL.subtract)
                else:
                    nc.vector.tensor_tensor(out=t1, in0=t1, in1=rBB, op=AL.subtract)
                    t2 = work.tile([128, SP], F16, name="t2g", tag="gelu_tmp2")
                    tt(t2, rCB, seqT[b][:, c, :], AL.mult, gp=False)
                    nc.vector.tensor_tensor(out=dst_big[:, c, :], in0=t1, in1=t2,
                                            op=AL.add)

        # ================= initial transform (per group) ====================
        pre_t = []
        for g in range(2):
            ps6i = psum.tile([66, SP], F32, name=f"ps_init{g}", tag="ps6", bufs=2)
            for b in (2 * g, 2 * g + 1):
                sA = work_big(f"sA{b}", tag="axT", bufs=2)
                nc.sync.dma_start(out=sA,
                                  in_=seqT_in.ap()[b].rearrange("c p i -> p c i"))
                pre = work_big(f"pre{b}", tag="preT", bufs=4)
                for c in range(DC):
                    ps = psum.tile([128, SP], F32, name=f"ps_pre{b}{c}",
                                   tag="psmm", bufs=2)
                    mm(ps, [itW_t[k][:, c * 128:(c + 1) * 128] for k in range(2)],
                       [sA[:, 0, :], sA[:, 1, :]])
                    nc.scalar.activation(out=pre[:, c, :], in_=ps, func=AF.Copy)
                if b % 2 == 0:
                    nc.vector.memset(ps6i, 0.0)
                stat_mms(ps6i, b, pre, last=(b % 2 == 1))
                pre_t.append(pre)
            _, rstd, mr = ln_rows2(ps6i, want_tsc=False)
            rAi = row4(f"rAi{g}", F16)[0:2]
            nc.vector.tensor_tensor(out=rAi, in0=rstd, in1=mask16[g],
                                    op=AL.mult)
            nc.sync.dma_start(out=r2i_d[4 * g:4 * g + 2], in_=rAi)
            rBi = row4(f"rBi{g}", F16)[0:2]
            nc.vector.tensor_tensor(out=rBi, in0=mr, in1=mask16[g],
                                    op=AL.mult)
            nc.sync.dma_start(out=r2i_d[4 * g + 2:4 * g + 4], in_=rBi)
        for b in range(NB):
            g, gb = b // 2, b % 2
            rAB = bcast_read(r2i_d[4 * g + gb:4 * g + gb + 1, :], f"rAB0_{b}")
            rBB = bcast_read(r2i_d[4 * g + 2 + gb:4 * g + 3 + gb, :], f"rBB0_{b}")
            apply_ln_gated(seqT[b], pre_t[b], rAB, rBB, None, b)

        # ================= per-group row tail ===============================
        def emit_tail2(g, ps6, last):
            dsl = slice(2 * g, 2 * g + 2)
            tsc2, rstd, mr = ln_rows2(ps6)
            masked = row4("msk", F16)[0:2]
            nc.vector.tensor_tensor(out=masked, in0=tsc2, in1=selp16[g],
                                    op=AL.mult)
            mx = tiny4("mx")[0:2]
            nc.vector.tensor_reduce(out=mx, in_=masked,
                                    axis=mybir.AxisListType.X, op=AL.max)
            negmx = tiny4("negmx")[0:2]
            nc.vector.tensor_scalar(out=negmx, in0=mx, scalar1=0.0,
                                    scalar2=-1.0, op0=AL.max, op1=AL.mult)
            et = row4("et", F16)[0:2]
            nc.scalar.activation(out=et, in_=tsc2, func=AF.Exp, bias=negmx)
            nc.vector.tensor_tensor(out=et, in0=et, in1=selp16[g], op=AL.mult)
            en = tiny4("en")[0:2]
            nc.scalar.activation(out=en, in_=negmx, func=AF.Exp)
            nc.vector.tensor_scalar(out=en, in0=en, scalar1=EPS, scalar2=None,
                                    op0=AL.add)
            # den in fp16, clamped well above fp16 underflow: tp at clamped
            # positions has et ~ 0 there anyway (selp mask), so tp stays 0
            den = row4("den", F16)[0:2]
            nc.vector.tensor_scalar(out=den, in0=et, scalar1=en, scalar2=6e-5,
                                    op0=AL.add, op1=AL.max)
            dei = row4("dei", F16)[0:2]
            recip(dei, den)
            tp = row4("tp", F16)[0:2]
            nc.vector.tensor_tensor(out=tp, in0=et, in1=dei, op=AL.mult)

            # LN-apply rows -> r3_d[6g:6g+6] (rA {gb}, rB {2+gb}, rC {4+gb})
            tpm = row4("tpm", F16)[0:2]
            nc.vector.tensor_tensor(out=tpm, in0=tp, in1=mask16[g], op=AL.mult)
            rAx = row4("rAx", F16)[0:2]
            nc.vector.tensor_tensor(out=rAx, in0=tpm, in1=rstd, op=AL.mult)
            nc.sync.dma_start(out=r3_d[6 * g:6 * g + 2], in_=rAx)
            rBx = row4("rBx", F16)[0:2]
            nc.vector.tensor_tensor(out=rBx, in0=tpm, in1=mr, op=AL.mult)
            nc.sync.dma_start(out=r3_d[6 * g + 2:6 * g + 4], in_=rBx)
            rCx = row4("rCx", F16)[0:2]
            nc.vector.tensor_tensor(out=rCx, in0=mask16[g], in1=tpm,
                                    op=AL.subtract)
            nc.sync.dma_start(out=r3_d[6 * g + 4:6 * g + 6], in_=rCx)

            if not last:
                nc.sync.dma_start(out=ltp4_d[dsl], in_=tp)
                # deact scan + active update
                nc.vector.tensor_scalar(out=nap2[g][:, 1:SP + 1], in0=a_row2[g],
                                        scalar1=-1.0, scalar2=1.0,
                                        op0=AL.mult, op1=AL.add)
                nc.vector.tensor_copy(out=tpp2[g][:, 1:SP + 1], in_=tp)
                # (fp16 4x copy; deact scan reads fp16 data1, fp32 coeff)
                u = row4("u")[0:2]
                nc.vector.tensor_tensor_scan(
                    out=u[:, ::-1], data0=nap2[g][:, SPP - 1:1:-1],
                    data1=tpp2[g][:, SPP - 1:1:-1], initial=0.0,
                    op0=AL.mult, op1=AL.add)
                nd = row4("nd")[0:2]
                nc.vector.tensor_tensor(out=nd, in0=a_row2[g], in1=u, op=AL.mult)
                nc.vector.tensor_scalar(out=nd, in0=nd, scalar1=-1.0, scalar2=1.0,
                                        op0=AL.mult, op1=AL.add)
                nc.vector.tensor_tensor(out=nd, in0=a_row2[g], in1=nd,
                                        op=AL.mult)
                nc.vector.tensor_scalar(out=nd, in0=nd, scalar1=0.0, scalar2=1.0,
                                        op0=AL.max, op1=AL.min)
                nc.vector.tensor_tensor(out=a_row2[g], in0=nd, in1=mask2[g],
                                        op=AL.mult)
                a16 = row4("a16", F16)[0:2]
                nc.vector.tensor_copy(out=a16, in_=a_row2[g])
                nc.sync.dma_start(out=a4_d[dsl], in_=a16)
                nar = row4("nar")[0:2]
                nc.vector.tensor_scalar(out=nar, in0=a_row2[g], scalar1=-1.0,
                                        scalar2=1.0, op0=AL.mult, op1=AL.add)
                nc.sync.dma_start(out=na4_d[dsl], in_=nar)

        # ================= deferred per-batch back half =====================
        # w2 -> gated sum -> LN stats.  Emitted one batch late so the DVE
        # gating never waits on the PE/ACT w1->gelu->w2->sigmoid pipeline.
        def emit_back(b, ps12, lcT, interT, g, gb, s):
            comp = compS[b]
            parT = work_big(f"parT{b}", tag="gpar", bufs=3)
            inter_lhsT = [interT[:, hk, :] for hk in range(8)]
            for gg in [3, 0, 1, 2]:
                for c in range(DC):
                    cc = gg * DC + c
                    ps = psum.tile([128, SP], F32, name=f"ps_w2{b}{cc}",
                                   tag="psmm", bufs=2)
                    if W2_FP8:
                        mm_dr(ps, w2W_t, [interT[:, 2 * p:2 * p + 2, :]
                                          for p in range(4)], cc * 128, 128)
                        osc = 1.0 / WSCALE
                    else:
                        mm(ps, [w2W_t[hk][:, cc * 128:(cc + 1) * 128]
                                for hk in range(8)], inter_lhsT)
                        osc = 1.0
                    if gg == 3:
                        nc.scalar.activation(out=parT[:, c, :], in_=ps,
                                             func=AF.Copy, scale=osc)
                    else:
                        gate = work.tile([128, SP], F16, name=f"gate{b}",
                                         tag="gate", bufs=4)
                        nc.scalar.activation(out=gate, in_=ps, func=AF.Sigmoid,
                                             bias=0.0, scale=osc)
                        srcx = [lcT, seqT[b], parT][gg]
                        if gg == 0:
                            nc.vector.tensor_tensor(out=comp[:, c, :], in0=gate,
                                                    in1=srcx[:, c, :],
                                                    op=AL.mult)
                        else:
                            gm = work.tile([128, SP], F16, name=f"gm{b}",
                                           tag="gelu_tmp2")
                            tt(gm, gate, srcx[:, c, :], AL.mult, gp=True)
                            nc.vector.tensor_tensor(out=comp[:, c, :],
                                                    in0=comp[:, c, :],
                                                    in1=gm, op=AL.add)
            stat_mms(ps12, b, comp, last=(gb == 1))
            if gb == 1:
                pending_tails.append((g, ps12, s == n_steps - 1))

        # ================= main steps =======================================
        pending_tails = []
        pending_back = []
        for s in range(n_steps):
            ps6g = [psum.tile([66, SP], F32, name=f"ps6_{s}{g}", tag="ps6", bufs=2)
                    for g in range(2)]
            for b in range(NB):
                g, gb = b // 2, b % 2
                ps12 = ps6g[g]
                # ---- seq update from previous step's rows ----
                if s > 0:
                    rAB = bcast_read(r3_d[6 * g + gb:6 * g + gb + 1, :], f"rAB{b}")
                    rBB = bcast_read(r3_d[6 * g + 2 + gb:6 * g + 3 + gb, :],
                                     f"rBB{b}")
                    rCB = bcast_read(r3_d[6 * g + 4 + gb:6 * g + 5 + gb, :],
                                     f"rCB{b}")
                    apply_ln_gated(seqT[b], compS[b], rAB, rBB, rCB, b)

                # ---- phase A: broadcasts + base ----
                if s == 0:
                    aB = naB = None  # active == mask: scans are pure shifts
                else:
                    aB = bcast_read(a4_d[b:b + 1, :], f"aB{b}")
                    ltpB = bcast_read(ltp4_d[b:b + 1, :], f"ltpB{b}")
                    naB = naBS[b]
                    nc.sync.dma_start(out=naB[:, 1:SP + 1],
                                      in_=_bcast_ap(na4_d[b:b + 1, :]))
                baseT = work_big(f"baseT{b}", tag="baseT", bufs=2)
                if s == 0:
                    for c in range(DC):
                        nc.vector.tensor_scalar(
                            out=baseT[:, c, :], in0=seqT[b][:, c, :],
                            scalar1=noc[:, c:c + 1], scalar2=None, op0=AL.add)
                else:
                    for c in range(DC):
                        tfc = work.tile([128, SP], F16, name=f"tfc{b}",
                                        tag="gate", bufs=4)
                        nc.vector.tensor_scalar(
                            out=tfc, in0=ltpB, scalar1=ymnc[:, c:c + 1],
                            scalar2=noc[:, c:c + 1], op0=AL.mult, op1=AL.add)
                        nc.vector.tensor_tensor(
                            out=baseT[:, c, :], in0=tfc, in1=seqT[b][:, c, :],
                            op=AL.add)

                # ---- phase B: 5 scans ----
                def fill_ax(axt, src_big):
                    for c in range(DC):
                        tt(axt[:, c, 1:SP + 1], aB, src_big[:, c, :],
                           AL.mult, gp=(GP_LVL >= 2))

                def shift_copy(dst, src_big, sh):
                    """dst[i] = src[i-sh] (zeros shifted in); s==0 fast path
                    where every neighbor scan degenerates to a shift.  Pad
                    positions differ from the true recurrence but every
                    consumer there is masked (selp/mask zero)."""
                    for c in range(DC):
                        if sh > 0:
                            nc.vector.memset(dst[:, c, 0:sh], 0.0)
                            nc.vector.tensor_copy(out=dst[:, c, sh:SP],
                                                  in_=src_big[:, c, 0:SP - sh])
                        else:
                            nc.vector.memset(dst[:, c, SP + sh:SP], 0.0)
                            nc.vector.tensor_copy(out=dst[:, c, 0:SP + sh],
                                                  in_=src_big[:, c, -sh:SP])

                # lcT first: unblocks w1/w2 on PE while the l1/l2 chain runs
                lcT = work_big(f"lcT{b}", tag="lcT", bufs=2)
                if s == 0:
                    axB = None
                    shift_copy(lcT, seqT[b], 1)
                else:
                    axB = axS[b]
                    fill_ax(axB, seqT[b])
                    for c in range(DC):
                        scan_fwd(lcT[:, c, :], naB, axB[:, c])

                # deferred row tail of a completed group, one extra block
                # late so its bounce-DMA latency is fully hidden
                if gb == 1 and pending_tails:
                    emit_tail2(*pending_tails.pop(0))

                # previous batch's w2/gating: its gates are ready by now, and
                # its PE work stays directly behind that batch's score
                if pending_back:
                    emit_back(*pending_back.pop(0))

                # w1 -> gelu -> interT issued early on PE
                interT = work.tile([128, 8, SP], FP8 if W2_FP8 else F16,
                                   name=f"interT{b}", tag="interT", bufs=2)
                if W1_FP8:
                    # fp8 copies on the (otherwise idle) Pool engine; lcT
                    # itself stays fp16 so the composer gating is unpolluted
                    lc8 = work_big(f"lc8{b}", tag="lc8", dtype=FP8, bufs=2)
                    seq8 = work_big(f"seq8{b}", tag="seq8", dtype=FP8, bufs=2)
                    for c in range(DC):
                        nc.gpsimd.tensor_tensor(out=lc8[:, c, :],
                                                in0=lcT[:, c, :], in1=ones16,
                                                op=AL.mult)
                        nc.gpsimd.tensor_tensor(out=seq8[:, c, :],
                                                in0=seqT[b][:, c, :], in1=ones16,
                                                op=AL.mult)
                    for hk in range(8):
                        ps = psum.tile([128, SP], F32, name=f"ps_w1{b}{hk}",
                                       tag="psmm", bufs=2)
                        mm_dr(ps, [w1W_t[0], w1W_t[1]], [lc8, seq8],
                              hk * 128, 128)
                        gelu_act(interT[:, hk, :], ps, scale=1.0 / WSCALE)
                else:
                    cc_rhs = [lcT[:, 0, :], lcT[:, 1, :],
                              seqT[b][:, 0, :], seqT[b][:, 1, :]]
                    for hk in range(8):
                        ps = psum.tile([128, SP], F32, name=f"ps_w1{b}{hk}",
                                       tag="psmm", bufs=2)
                        mm(ps, [w1W_t[k][:, hk * 128:(hk + 1) * 128]
                                for k in range(4)], cc_rhs)
                        gelu_act(interT[:, hk, :], ps)

                l1T = work_big(f"l1T{b}", tag="l1T", bufs=2)
                r1T = work_big(f"r1T{b}", tag="r1T", bufs=2)
                l2T = work_big(f"l2T{b}", tag="l2T", bufs=2)
                r2T = work_big(f"r2T{b}", tag="r2T", bufs=2)
                if s == 0:
                    shift_copy(l1T, baseT, 1)
                    shift_copy(r1T, baseT, -1)
                    shift_copy(l2T, baseT, 2)
                    shift_copy(r2T, baseT, -2)
                else:
                    fill_ax(axB, baseT)
                    for c in range(DC):
                        scan_fwd(l1T[:, c, :], naB, axB[:, c])
                        scan_bwd(r1T[:, c, :], naB, axB[:, c])
                    ax2 = ax2S[b]
                    fill_ax(ax2, l1T)
                    for c in range(DC):
                        scan_fwd(l2T[:, c, :], naB, ax2[:, c])
                    fill_ax(ax2, r1T)
                    for c in range(DC):
                        scan_bwd(r2T[:, c, :], naB, ax2[:, c])

                # ---- phase C: conv (transposed) + score ----
                piece_order = [(2, baseT), (1, l1T), (3, r1T), (0, l2T), (4, r2T)]
                gT = work_big(f"gT{b}", tag="gpar", bufs=3)
                for c in range(DC):
                    ps = psum.tile([128, SP], F32, name=f"ps_cv{b}{c}", tag="psmm", bufs=2)
                    lhsT, rhs = [], []
                    for w, piece in piece_order:
                        for ci in range(DC):
                            lhsT.append(convW_t[w * DC + ci][:, c * 128:(c + 1) * 128])
                            rhs.append(piece[:, ci, :])
                    mm(ps, lhsT, rhs)
                    gelu_act(gT[:, c, :], ps)
                # score -> stats psum rows 64-65 via one-hot scW columns
                if gb == 0:
                    nc.vector.memset(ps12, 0.0)
                mm_acc(ps12, [scw6_t[b][:, c, :] for c in range(DC)],
                       [gT[:, c, :] for c in range(DC)])

                pending_back.append((b, ps12, lcT, interT, g, gb, s))

        while pending_back:
            emit_back(*pending_back.pop(0))
        while pending_tails:
            emit_tail2(*pending_tails.pop(0))

        # final seq update
        for b in range(NB):
            g, gb = b // 2, b % 2
            rAB = bcast_read(r3_d[6 * g + gb:6 * g + gb + 1, :], f"rABf{b}")
            rBB = bcast_read(r3_d[6 * g + 2 + gb:6 * g + 3 + gb, :], f"rBBf{b}")
            rCB = bcast_read(r3_d[6 * g + 4 + gb:6 * g + 5 + gb, :], f"rCBf{b}")
            apply_ln_gated(seqT[b], compS[b], rAB, rBB, rCB, b)

        # ---------------- output ------------------------------------------
        for b in range(NB):
            for c in range(DC):
                nc.sync.dma_start(out=out_dram.ap()[b, c],
                                  in_=seqT[b][:, c, 0:S2])
    return nc


def _host_prep(inputs):
    f32 = np.float32
    f16 = np.float16
    seq = np.asarray(inputs["sequence"], f32)
    im = np.asarray(inputs["input_mask"], f32)
    START = np.asarray(inputs["START"], f32)
    END = np.asarray(inputs["END"], f32)
    yes_t = np.asarray(inputs["yes_t"], f32).reshape(-1)
    no_t = np.asarray(inputs["no_t"], f32).reshape(-1)
    N, S, Dd = seq.shape
    assert (N, S, Dd) == (32, 512, 256), (N, S, Dd)

    ones = np.ones((N, 1, 1), f32)
    zeros = np.zeros((N, 1, 1), f32)
    mask0 = np.concatenate([ones, im], 1)
    mask_no_end = np.concatenate([mask0, zeros], 1)
    mask_yes_end = np.concatenate([ones, mask0], 1)
    END_mask = mask_yes_end - mask_no_end
    seqA = np.concatenate([np.broadcast_to(START, (N, 1, Dd)), seq,
                           np.zeros((N, 1, Dd), f32)], 1)
    seqA = (END_mask * END + (1.0 - END_mask) * seqA).astype(f32)
    mask = mask_yes_end
    mask_no_start = np.concatenate([zeros, mask[:, 1:]], 1)
    last_tok = np.concatenate([END_mask[:, 1:], zeros], 1)
    selp = (mask_no_start * mask_no_end * (1.0 - last_tok)).astype(f32)

    seqT = np.zeros((N, DC, 128, SP), f16)
    for c in range(DC):
        seqT[:, c, :, :S2] = seqA[:, :, c * 128:(c + 1) * 128].transpose(0, 2, 1)
    maskP = np.zeros((N, SP), f32)
    maskP[:, :S2] = mask[:, :, 0]
    selpP = np.zeros((N, SP), f32)
    selpP[:, :S2] = selp[:, :, 0]

    def chunk_col(v, nch):
        return np.ascontiguousarray(np.asarray(v, f32).reshape(nch, 128).T)

    scWc = chunk_col(np.asarray(inputs["scW"], f32).reshape(-1), DC)
    scw6 = np.zeros((NB, 128, DC, 66), f16)
    for b in range(NB):
        for c in range(DC):
            scw6[b, :, c, 64 + b % 2] = scWc[:, c]

    host = {
        "seqT": seqT, "mask": maskP, "selp": selpP,
        "a16": maskP.astype(f16), "na32": (1.0 - maskP).astype(f32),
        "itW": np.asarray(inputs["itW"], f16),
        "convW": np.asarray(inputs["convW"], f16),
        "scw6": scw6,
        "noc": chunk_col(no_t, DC),
        "ymnc": chunk_col(yes_t - no_t, DC),
    }

    def fp8_pairs(w, npairs, M):
        import ml_dtypes
        w = np.asarray(w, f32) * WSCALE
        out = np.zeros((npairs, 128, 2, M), np.uint8)
        for p in range(npairs):
            for j in range(2):
                blk = w[(2 * p + j) * 128:(2 * p + j + 1) * 128, :]
                out[p, :, j, :] = blk.astype(ml_dtypes.float8_e4m3fn).view(np.uint8)
        return out

    if W1_FP8:
        host["w1p"] = fp8_pairs(inputs["w1W"], 2, H)
    else:
        host["w1W"] = np.asarray(inputs["w1W"], f16)
    if W2_FP8:
        host["w2p"] = fp8_pairs(inputs["w2W"], 4, 4 * D)
    else:
        host["w2W"] = np.asarray(inputs["w2W"], f16)
    return host


_PROG_CACHE = {}


def kernel(**inputs):
    global LAST_EXEC_NS, LAST_RES, LAST_NC
    n_steps = int(inputs["n_steps"])
    host = _host_prep(inputs)

    key = (n_steps, SIM, GP_LVL, W1_FP8, W2_FP8)
    if key not in _PROG_CACHE:
        _PROG_CACHE[key] = _build_program(n_steps)
    nc = _PROG_CACHE[key]
    LAST_NC = nc

    per_batch = {"seqT", "mask", "selp", "a16", "na32"}
    in_maps = []
    for k in range(NCORES):
        m = {}
        for name, arr in host.items():
            if name in per_batch:
                m[name] = np.ascontiguousarray(arr[k * NB:(k + 1) * NB])
            else:
                m[name] = arr
        in_maps.append(m)

    if SIM:
        from concourse.bass_interp import CoreSim
        results = []
        for k in range(int(os.environ.get("CRVNN_SIM_CORES", "1"))):
            sim = CoreSim(nc)
            for name, v in in_maps[k].items():
                sim.tensor(name)[:] = v
            sim.simulate()
            results.append(np.array(sim.tensor("out")))
    else:
        from concourse.bass_utils import run_bass_kernel_spmd
        if not getattr(nc, "_waitfix_done", False):
            _split_multiwaits(nc)
            nc._waitfix_done = True
        res = run_bass_kernel_spmd(nc, in_maps, list(range(NCORES)), trace=TRACE)
        LAST_EXEC_NS = res.exec_time_ns
        LAST_RES = res
        results = [res.results[k]["out"] for k in range(NCORES)]

    full = np.zeros((32, S2, D), np.float32)
    for k, o in enumerate(results):
        for b in range(NB):
            for c in range(DC):
                full[k * NB + b, :, c * 128:(c + 1) * 128] = \
                    o[b, c].astype(np.float32).T
    return full
